# revision 22
# baseline (speedup 1.0000x reference)
"""Self-contained Trainium2 Bass kernel for the 4-layer GraphSAGE GNN
(nn_EnhancedClassifier): kernel(**inputs) -> np.ndarray [100000] f32.

Runs SPMD on 8 NeuronCores via run_bass_kernel_spmd.

v2 strategy: dst-partition nodes across 8 cores. Per core, edges are
sorted by (src-quarter stream, dst_block) and packed into 4 per-chunk
tile streams with unit-anchored scheduling: groups of UNIT dst blocks
share a start offset = cumulative max edge count over cores (keeps the
SPMD program uniform with only ~5% pad, vs 25% for per-block max), and
tiles at block boundaries are shared by adjacent blocks (the one-hot
zeroes foreign edges). x is zero-padded to 128 feats bf16 so all 4
layers gather 256B rows. h_full is split into 4 per-quarter Shared DRAM
tensors so each AllGather quarter unblocks that chunk's gathers early.
The own-path h stays resident in SBUF between layers; PSUM->SBUF copies
run on the Scalar engine to keep DVE free for one-hot builds.
"""
import sys
sys.path.insert(0, '/opt/trn_rl_repo')
import numpy as np
import ml_dtypes
from concourse import bass, bacc, mybir, tile

BF16 = mybir.dt.bfloat16
F32 = mybir.dt.float32
FP8 = mybir.dt.float8e4
AF = mybir.ActivationFunctionType
ALU = mybir.AluOpType

NCORES = 8

# --- Patch Tile's DMASW lane assignment to be SWDGE-queue-aware: lane%4 must
# equal the instruction's queue_num or the runtime rejects the sem update.
import concourse.tile_sem_assignment as _tsa
from concourse import bass_isa as _bisa

if not getattr(_tsa, "_gnn_queue_patch", False):
    _orig_assign_tick = _tsa.TileClockTick._assign_tick

    def _assign_tick_qaware(self, inst):
        if isinstance(inst, mybir.InstDMAGatherAnt):
            q = inst.queue_num
            rot = self.__dict__.setdefault("_gnn_qrot", {})
            k = rot.get(q, 0)
            rot[q] = k ^ 1
            self.next_sw_dma_idx = q + 4 * k
        elif (isinstance(inst, _tsa.DMAInst)
              and inst.engine == mybir.EngineType.Pool
              and not isinstance(inst, _bisa.UserSyncedRemoteDMADescs)):
            rot = self.__dict__.setdefault("_gnn_qrot", {})
            k = rot.get(0, 0)
            rot[0] = k ^ 1
            self.next_sw_dma_idx = 4 * k
        return _orig_assign_tick(self, inst)

    _tsa.TileClockTick._assign_tick = _assign_tick_qaware
    _tsa._gnn_queue_patch = True

IN_F = 64
HID = 128
PAD_DSTLOC = 1000.0
NQ = 4              # chunk streams / AllGather quarters
import os as _os
WT = int(_os.environ.get("GNN_WT", "8"))   # tiles per dma_gather call
UNIT = 1            # dst blocks sharing one anchored stream offset
SPLIT_AG = int(_os.environ.get("GNN_SPLIT_AG", "1"))   # per-quarter AllGathers
# one-hot DMA batching: blocks are grouped until their span-columns reach
# GCAP, and each group is fetched with ONE dma_start -> ~GCAP*128/ncols-KB
# packets instead of 2.6KB per-block packets (packet setup dominated).
GCAP = int(_os.environ.get("GNN_GCAP", "96"))


class Cfg:
    def __init__(self, n_nodes):
        self.N = n_nodes
        self.NPC = n_nodes // NCORES
        assert self.NPC * NCORES == self.N
        self.B = (self.NPC + 127) // 128             # blocks per core
        self.ROWS = self.B * 128                     # padded rows per core
        self.GROWS = self.ROWS * NCORES
        # block-aligned quarters of each core's rows
        bq = self.B // NQ
        extra = self.B - bq * NQ
        self.qblocks = [bq + (1 if k < extra else 0) for k in range(NQ)]
        self.qrows = [q * 128 for q in self.qblocks]
        self.qstart = np.concatenate([[0], np.cumsum(self.qrows)]).astype(np.int64)
        self.chunk_rows = [NCORES * r for r in self.qrows]
        assert max(self.chunk_rows) <= 32768, "idx must fit int16"


def preprocess(cfg, x, edge_index, weights):
    src = edge_index[0].astype(np.int64)
    dst = edge_index[1].astype(np.int64)
    N, B = cfg.N, cfg.B

    deg = np.bincount(dst, minlength=N).astype(np.float32)
    deginv = 1.0 / np.maximum(deg, 1.0)

    s_core = src // cfg.NPC
    s_loc = src % cfg.NPC
    s_q = np.searchsorted(cfg.qstart[1:], s_loc, side='right')
    qrows_a = np.asarray(cfg.qrows, np.int64)
    s_row = s_core * qrows_a[s_q] + (s_loc - cfg.qstart[s_q])   # chunk-relative

    d_core = dst // cfg.NPC
    d_loc = dst % cfg.NPC
    d_block = d_loc // 128

    # within each (core, stream, dst-block) run, order slots by ascending src
    # row: the one-hot encodes slot->dst anyway, and ascending gather
    # addresses give the HBM better locality than dst-sorted (random) reads.
    order = np.lexsort((s_row, d_block, s_q, d_core))
    s_row_s = s_row[order]
    d_loc_s = d_loc[order]
    key = (d_core[order] * NQ + s_q[order]) * B + d_block[order]
    gstart = np.searchsorted(key, np.arange(NCORES * NQ * B + 1))

    # per (core, q, b) counts
    cnt = (gstart[1:] - gstart[:-1]).reshape(NCORES, NQ, B)

    # ---- unit-anchored stream scheduling (uniform across cores)
    NU = (B + UNIT - 1) // UNIT
    S = np.zeros((NQ, NU + 1), np.int64)            # unit start slots per stream
    ucnt_max = np.zeros((NQ, NU), np.int64)
    for q in range(NQ):
        for u in range(NU):
            b0, b1 = u * UNIT, min(B, (u + 1) * UNIT)
            ucnt_max[q, u] = cnt[:, q, b0:b1].sum(axis=1).max()
        S[q, 1:] = np.cumsum(ucnt_max[q])
    NTq = [int((S[q, NU] + 127) // 128) for q in range(NQ)]
    qtile0 = np.concatenate([[0], np.cumsum(NTq)]).astype(np.int64)
    NT = int(qtile0[-1])
    SLOTS = NT * 128

    # per-core block positions within streams
    pos0 = np.zeros((NCORES, NQ, B), np.int64)
    pos1 = np.zeros((NCORES, NQ, B), np.int64)
    for q in range(NQ):
        for u in range(NU):
            b0, b1 = u * UNIT, min(B, (u + 1) * UNIT)
            run = np.cumsum(
                np.concatenate([np.zeros((NCORES, 1), np.int64),
                                cnt[:, q, b0:b1]], axis=1), axis=1)
            pos0[:, q, b0:b1] = S[q, u] + run[:, :-1]
            pos1[:, q, b0:b1] = S[q, u] + run[:, 1:]

    # union spans per (b, q) across cores
    uspans = []
    for b in range(B):
        sp = []
        for q in range(NQ):
            mask = pos1[:, q, b] > pos0[:, q, b]
            if mask.any():
                t0 = int(pos0[mask, q, b].min() // 128)
                t1 = int(-(-pos1[mask, q, b].max() // 128))
                sp.append((q, t0, t1))
        uspans.append(sp)
    cols_b = [sum(t1 - t0 for (_, t0, t1) in uspans[b]) for b in range(B)]
    COLS = int(sum(cols_b))
    col0_b = np.concatenate([[0], np.cumsum(cols_b)]).astype(np.int64)
    MAXSPAN = max(cols_b)

    # gather calls: interleave streams window-major
    calls = []
    for lo in range(0, max(NTq), WT):
        for q in range(NQ):
            if lo < NTq[q]:
                calls.append((q, lo, min(WT, NTq[q] - lo)))

    # one-hot load groups: consecutive blocks packed until GCAP span-columns
    groups = []           # (b0, nblocks, col0, ncols)
    blk2grp = np.zeros(B, np.int64)
    b = 0
    while b < B:
        b0, ctot = b, 0
        while b < B and (b == b0 or ctot + cols_b[b] <= GCAP):
            ctot += cols_b[b]
            blk2grp[b] = len(groups)
            b += 1
        groups.append((b0, b - b0, int(col0_b[b0]), ctot))
    assert max(g[3] for g in groups) <= max(GCAP, max(cols_b))

    layout = dict(NTq=NTq, qtile0=qtile0, NT=NT, SLOTS=SLOTS, uspans=uspans,
                  cols_b=cols_b, col0_b=col0_b, COLS=COLS, MAXSPAN=MAXSPAN,
                  calls=calls, groups=groups, blk2grp=blk2grp)

    # ---- weights / constants
    W = {k: np.asarray(v) for k, v in weights.items()}

    def padw(w):
        out = np.zeros((HID, HID), np.float32)
        out[:w.shape[0], :w.shape[1]] = w
        return out.astype(ml_dtypes.bfloat16)

    wcast = {
        "Wl1": padw(W["Wl1"]), "Wr1": padw(W["Wr1"]), "Wres": padw(W["Wres"]),
        "Wl2": W["Wl2"].astype(ml_dtypes.bfloat16),
        "Wr2": W["Wr2"].astype(ml_dtypes.bfloat16),
        "Wl3": W["Wl3"].astype(ml_dtypes.bfloat16),
        "Wr3": W["Wr3"].astype(ml_dtypes.bfloat16),
        "Wl4": W["Wl4"].astype(ml_dtypes.bfloat16),
        "Wr4": W["Wr4"].astype(ml_dtypes.bfloat16),
    }
    brow = {k: W[k].reshape(1, -1).astype(ml_dtypes.bfloat16)
            for k in ["b1", "bres", "b2", "b3", "b4"]}
    ln_g = W["ln_g"].astype(np.float32)
    ln_b = W["ln_b"].astype(np.float32)
    ln_identity = bool(np.all(ln_g == 1.0) and np.all(ln_b == 0.0))
    ln_g_rep = np.broadcast_to(ln_g, (128, HID)).copy()
    ln_b_rep = np.broadcast_to(ln_b, (128, HID)).copy()

    iota_bf = np.broadcast_to(np.arange(128, dtype=np.float32), (128, 128)).astype(
        ml_dtypes.bfloat16).copy()
    ones_row = np.ones((1, 128), ml_dtypes.bfloat16)
    ident_bf = np.eye(128, dtype=ml_dtypes.bfloat16)

    # x zero-padded to 128 feats bf16, laid out [chunk q][core][qrows]
    x_cast = np.asarray(x, np.float32).astype(ml_dtypes.bfloat16)
    xq = []
    for q in range(NQ):
        xb = np.zeros((cfg.chunk_rows[q], HID), ml_dtypes.bfloat16)
        for c in range(NCORES):
            lo = c * cfg.NPC + cfg.qstart[q]
            n = int(min(cfg.qrows[q], max(0, cfg.NPC - cfg.qstart[q])))
            if n > 0:
                xb[c * cfg.qrows[q]: c * cfg.qrows[q] + n, :IN_F] = x_cast[lo:lo + n]
        xq.append(xb)

    # ---- per-core inputs
    in_maps = []
    for c in range(NCORES):
        idx_lin = np.zeros(SLOTS, np.int16)
        for q in range(NQ):
            g0 = (c * NQ + q) * B
            for b in range(B):
                lo_e, hi_e = gstart[g0 + b], gstart[g0 + b + 1]
                n = int(hi_e - lo_e)
                if n == 0:
                    continue
                s0 = int(pos0[c, q, b])
                gslot = qtile0[q] * 128 + s0
                idx_lin[gslot:gslot + n] = s_row_s[lo_e:hi_e].astype(np.int16)
        # host-built one-hot, fp8: oh_all[p, col*128 + j] = 1 iff the edge at
        # slot (tile(col), p) belongs to block(col) and has dst j (local)
        oh_all = np.zeros((128, COLS * 128), ml_dtypes.float8_e4m3fn)
        col = 0
        for b in range(B):
            for (q, t0, t1) in uspans[b]:
                g0 = (c * NQ + q) * B
                lo_e, hi_e = gstart[g0 + b], gstart[g0 + b + 1]
                n = int(hi_e - lo_e)
                if n:
                    s0 = int(pos0[c, q, b])
                    dl = d_loc_s[lo_e:hi_e] - b * 128
                    sl = np.arange(s0, s0 + n)
                    tt = sl // 128 - t0
                    pp = sl % 128
                    oh_all[pp, (col + tt) * 128 + dl] = 1.0
                col += (t1 - t0)
        assert col == COLS

        idx_pk = idx_lin.reshape(SLOTS // 16, 16).T
        idx_pk = np.tile(idx_pk, (8, 1))

        dinv_col = np.ones((128, B), np.float32)
        basec = c * cfg.NPC
        for b in range(B):
            n_real = min(128, cfg.NPC - b * 128)
            dinv_col[:n_real, b] = deginv[basec + b * 128: basec + b * 128 + n_real]

        x_own = np.zeros((cfg.ROWS, HID), ml_dtypes.bfloat16)
        x_own[:cfg.NPC, :IN_F] = x_cast[c * cfg.NPC:(c + 1) * cfg.NPC]

        m = {
            "idx16": np.ascontiguousarray(idx_pk),
            "oh_all": oh_all,
            "deginv": dinv_col,
            "x_own": x_own,
            "ones_row": ones_row,
            "ident": ident_bf,
            "ln_g_rep": ln_g_rep,
            "ln_b_rep": ln_b_rep,
        }
        for q in range(NQ):
            m[f"x_q{q}"] = xq[q]
        m.update(wcast)
        m.update(brow)
        in_maps.append(m)

    return in_maps, layout, ln_identity


def build_program(cfg, layout, ln_identity):
    B, ROWS = cfg.B, cfg.ROWS
    COLS, SLOTS = layout["COLS"], layout["SLOTS"]
    uspans, col0_b = layout["uspans"], layout["col0_b"]
    qtile0, calls = layout["qtile0"], layout["calls"]
    MAXSPAN = layout["MAXSPAN"]
    nc = bacc.Bacc("TRN2", target_bir_lowering=False, debug=False,
                   num_devices=NCORES, num_swdge_queues=4)

    x_q = [nc.dram_tensor(f"x_q{q}", [cfg.chunk_rows[q], HID], BF16,
                          kind="ExternalInput") for q in range(NQ)]
    x_own = nc.dram_tensor("x_own", [ROWS, HID], BF16, kind="ExternalInput")
    idx_d = nc.dram_tensor("idx16", [128, SLOTS // 16], mybir.dt.int16,
                           kind="ExternalInput")
    oh_d = nc.dram_tensor("oh_all", [128, COLS * 128], FP8, kind="ExternalInput")
    deginv_d = nc.dram_tensor("deginv", [128, B], F32, kind="ExternalInput")
    ones_d = nc.dram_tensor("ones_row", [1, 128], BF16, kind="ExternalInput")
    ident_d = nc.dram_tensor("ident", [128, 128], BF16, kind="ExternalInput")
    lng_d = nc.dram_tensor("ln_g_rep", [128, HID], F32, kind="ExternalInput")
    lnb_d = nc.dram_tensor("ln_b_rep", [128, HID], F32, kind="ExternalInput")
    wd = {k: nc.dram_tensor(k, [HID, HID], BF16, kind="ExternalInput")
          for k in ["Wl1", "Wr1", "Wres", "Wl2", "Wr2", "Wl3", "Wr3"]}
    wd["Wl4"] = nc.dram_tensor("Wl4", [HID, 1], BF16, kind="ExternalInput")
    wd["Wr4"] = nc.dram_tensor("Wr4", [HID, 1], BF16, kind="ExternalInput")
    bd = {k: nc.dram_tensor(k, [1, HID], BF16, kind="ExternalInput")
          for k in ["b1", "bres", "b2", "b3"]}
    bd["b4"] = nc.dram_tensor("b4", [1, 1], BF16, kind="ExternalInput")

    out_d = nc.dram_tensor("out", [ROWS], F32, kind="ExternalOutput")
    rg = [list(range(NCORES))]

    with tile.TileContext(nc) as tc:
        with (
            tc.tile_pool(name="dramp", bufs=1, space="DRAM") as dramp,
            tc.tile_pool(name="const", bufs=1) as constp,
            tc.tile_pool(name="meta", bufs=1) as metap,
            tc.tile_pool(name="gpool", bufs=10) as gpool,
            tc.tile_pool(name="ohpool", bufs=3) as ohpool,
            tc.tile_pool(name="spool", bufs=4) as spool,
            tc.tile_pool(name="hres", bufs=1) as hresp,
            tc.tile_pool(name="outp", bufs=1) as outp,
            tc.tile_pool(name="ps", bufs=2, space="PSUM") as ps,
        ):
            if SPLIT_AG:
                # per-quarter local h and Shared gathered h (3 layers x NQ)
                h_own = [[dramp.tile([cfg.qrows[k], HID], BF16,
                                     tag=f"h_own{l}_{k}", name=f"h_own{l}_{k}")
                          for k in range(NQ)] for l in range(3)]
                h_full = [[dramp.tile([cfg.chunk_rows[k], HID], BF16,
                                      tag=f"h_full{l}_{k}", name=f"h_full{l}_{k}",
                                      addr_space="Shared") for k in range(NQ)]
                          for l in range(3)]
            else:
                h_own1 = [dramp.tile([ROWS, HID], BF16, tag=f"h_own{l}",
                                     name=f"h_own{l}") for l in range(3)]
                h_full1 = [dramp.tile([cfg.GROWS, HID], BF16, tag=f"h_full{l}",
                                      name=f"h_full{l}", addr_space="Shared")
                           for l in range(3)]
            coff = np.concatenate(
                [[0], np.cumsum(cfg.chunk_rows)]).astype(np.int64)

            def h_src_aps(l):
                if SPLIT_AG:
                    return [h_full[l][k][:] for k in range(NQ)]
                return [h_full1[l][int(coff[k]):int(coff[k + 1]), :]
                        for k in range(NQ)]

            idx_t = metap.tile([128, SLOTS // 16], mybir.dt.int16)
            nc.sync.dma_start(out=idx_t[:], in_=idx_d[:])
            deginv_t = metap.tile([128, B], F32)
            nc.sync.dma_start(out=deginv_t[:], in_=deginv_d[:])
            ones_t = constp.tile([1, 128], BF16)
            nc.sync.dma_start(out=ones_t[:], in_=ones_d[:])
            ident_t = constp.tile([128, 128], BF16)
            nc.sync.dma_start(out=ident_t[:], in_=ident_d[:])
            eps_t = constp.tile([128, 1], F32)
            nc.vector.memset(eps_t[:], 1e-5)
            lng_t = constp.tile([128, HID], F32)
            nc.sync.dma_start(out=lng_t[:], in_=lng_d[:])
            lnb_t = constp.tile([128, HID], F32)
            nc.sync.dma_start(out=lnb_t[:], in_=lnb_d[:])
            w_t = {}
            for k, h in wd.items():
                w_t[k] = constp.tile(list(h.shape), BF16, tag=f"w_{k}", name=f"w_{k}")
                nc.sync.dma_start(out=w_t[k][:], in_=h[:])
            b_t = {}
            for k, h in bd.items():
                b_t[k] = constp.tile(list(h.shape), BF16, tag=f"b_{k}", name=f"b_{k}")
                nc.sync.dma_start(out=b_t[k][:], in_=h[:])

            out_sb = outp.tile([128, B], F32)
            hsb = [hresp.tile([128, B * HID], BF16, tag=f"hsb{i}",
                              name=f"hsb{i}") for i in range(2)]

            state = {"layer": 0}

            def new_layer(src_list):
                state["src"] = src_list
                state["G"] = {}
                state["nxt"] = 0
                state["cov"] = [0] * NQ
                state["layer"] += 1

            def issue_calls_until(need_q, need_t1):
                while state["cov"][need_q] < need_t1:
                    k = state["nxt"]
                    assert k < len(calls), (need_q, need_t1, state["cov"])
                    q, lo, nt = calls[k]
                    G = gpool.tile([128, WT * HID], BF16, tag="G",
                                   name=f"G_{state['layer']}_{k}")
                    base16 = (int(qtile0[q]) + lo) * 8
                    rows = nt * 128
                    nc.gpsimd.dma_gather(
                        out_ap=G[:, :nt * HID].rearrange(
                            "p (t e) -> p t e", e=HID),
                        in_ap=state["src"][q],
                        idxs_ap=idx_t[:, base16:base16 + rows // 16],
                        num_idxs=rows,
                        num_idxs_reg=rows,
                        elem_size=HID,
                        queue_num=k % 4,
                    )
                    state["G"][(q, lo // WT)] = G
                    state["cov"][q] = lo + nt
                    state["nxt"] = k + 1

            groups, blk2grp = layout["groups"], layout["blk2grp"]
            GMAX = max(g[3] for g in groups)

            def build_onehot(b):
                # batched one-hot fetch: one dma_start per block-group
                nb = layout["cols_b"][b]
                g = int(blk2grp[b])
                b0, nbk, c0, ncols = groups[g]
                if b == b0 and ncols > 0:
                    oh = ohpool.tile([128, GMAX * 128], FP8, tag="oh")
                    nc.sync.dma_start(out=oh[:, :ncols * 128],
                                      in_=oh_d[:, c0 * 128:(c0 + ncols) * 128])
                    state["ohg"] = oh
                if nb == 0:
                    return None, 0
                return state["ohg"], int(col0_b[b]) - c0

            def scatter(b, agg_psum):
                for (q, t0, t1) in uspans[b]:
                    issue_calls_until(q, t1)
                oh, coff = build_onehot(b)
                if oh is None:
                    return False
                n_mm = sum(t1 - t0 for (_, t0, t1) in uspans[b])
                col = coff
                for (q, t0, t1) in uspans[b]:
                    for t in range(t0, t1):
                        G = state["G"][(q, t // WT)]
                        off = (t % WT) * HID
                        nc.tensor.matmul(
                            agg_psum[:], lhsT=G[:, off:off + HID],
                            rhs=oh[:, col * 128:(col + 1) * 128],
                            start=(col == coff), stop=(col == coff + n_mm - 1))
                        col += 1
                return True

            qlast = np.cumsum(cfg.qblocks) - 1      # last block of each quarter

            def maybe_allgather(l, b):
                if SPLIT_AG:
                    for k in range(NQ):
                        if b == qlast[k]:
                            nc.gpsimd.collective_compute(
                                "AllGather", ALU.bypass, replica_groups=rg,
                                ins=[h_own[l][k][:]], outs=[h_full[l][k][:]])
                elif b == B - 1:
                    nc.gpsimd.collective_compute(
                        "AllGather", ALU.bypass, replica_groups=rg,
                        ins=[h_own1[l][:]], outs=[h_full1[l][:]])

            def write_h(l, b, src_ap):
                if SPLIT_AG:
                    k = int(np.searchsorted(qlast, b))
                    b0 = int(qlast[k]) - cfg.qblocks[k] + 1
                    ro = (b - b0) * 128
                    nc.sync.dma_start(out=h_own[l][k][ro:ro + 128, :], in_=src_ap)
                else:
                    nc.sync.dma_start(
                        out=h_own1[l][b * 128:(b + 1) * 128, :], in_=src_ap)

            # =================== Layer 1 ===================
            new_layer([x_q[q][:] for q in range(NQ)])
            for b in range(B):
                xblk = spool.tile([128, HID], BF16, tag="hblk")
                nc.sync.dma_start(out=xblk[:], in_=x_own[b * 128:(b + 1) * 128, :])
                xT_ps = ps.tile([HID, 128], BF16, tag="xT_ps", bufs=1)
                nc.tensor.transpose(xT_ps[:], xblk[:], ident_t[:])
                xT = spool.tile([HID, 128], BF16, tag="hT")
                nc.scalar.activation(xT[:], xT_ps[:], AF.Copy)

                agg_ps = ps.tile([HID, 128], F32, tag="agg", bufs=2)
                has_agg = scatter(b, agg_ps)
                aggT = spool.tile([HID, 128], BF16, tag="aggT")
                if has_agg:
                    nc.scalar.activation(aggT[:], agg_ps[:], AF.Copy)
                else:
                    nc.vector.memset(aggT[:], 0.0)

                zA = ps.tile([128, HID], F32, tag="zA", bufs=2)
                nc.tensor.matmul(zA[:], lhsT=aggT[:], rhs=w_t["Wl1"][:],
                                 start=True, stop=True)
                zB = ps.tile([128, HID], F32, tag="zB", bufs=2)
                nc.tensor.matmul(zB[:], lhsT=xT[:], rhs=w_t["Wr1"][:],
                                 start=True, stop=False)
                nc.tensor.matmul(zB[:], lhsT=ones_t[:], rhs=b_t["b1"][:],
                                 start=False, stop=True)
                res = ps.tile([128, HID], F32, tag="res", bufs=1)
                nc.tensor.matmul(res[:], lhsT=xT[:], rhs=w_t["Wres"][:],
                                 start=True, stop=False)
                nc.tensor.matmul(res[:], lhsT=ones_t[:], rhs=b_t["bres"][:],
                                 start=False, stop=True)

                sA = spool.tile([128, HID], F32, tag="sA")
                nc.vector.tensor_scalar(
                    out=sA[:], in0=zA[:], scalar1=deginv_t[:, b:b + 1],
                    scalar2=None, op0=ALU.mult)
                z = spool.tile([128, HID], F32, tag="z")
                nc.vector.tensor_tensor(out=z[:], in0=sA[:], in1=zB[:], op=ALU.add)

                # LayerNorm via fused bn_stats: one DVE pass for mean+var,
                # then one Scalar pass Relu(z*rstd - mu*rstd)
                st6 = spool.tile([128, 6], F32, tag="st6")
                nc.vector.bn_stats(st6[:], z[:])
                agr = spool.tile([128, 2], F32, tag="agr")
                nc.vector.bn_aggr(agr[:], st6[:])
                std = spool.tile([128, 1], F32, tag="std")
                nc.scalar.activation(std[:], agr[:, 1:2], AF.Sqrt, bias=eps_t[:])
                rstd = spool.tile([128, 1], F32, tag="rstd")
                nc.vector.reciprocal(rstd[:], std[:])
                nmu = spool.tile([128, 1], F32, tag="nmu")
                nc.vector.tensor_scalar(out=nmu[:], in0=agr[:, 0:1],
                                        scalar1=rstd[:], scalar2=-1.0,
                                        op0=ALU.mult, op1=ALU.mult)

                if ln_identity:
                    zr = spool.tile([128, HID], F32, tag="zr")
                    nc.scalar.activation(zr[:], z[:], AF.Relu, bias=nmu[:],
                                         scale=rstd[:])
                else:
                    zn = spool.tile([128, HID], F32, tag="zn")
                    nc.scalar.activation(zn[:], z[:], AF.Identity, bias=nmu[:],
                                         scale=rstd[:])
                    nc.vector.tensor_tensor(out=zn[:], in0=zn[:], in1=lng_t[:],
                                            op=ALU.mult)
                    nc.vector.tensor_tensor(out=zn[:], in0=zn[:], in1=lnb_t[:],
                                            op=ALU.add)
                    zr = spool.tile([128, HID], F32, tag="zr")
                    nc.vector.tensor_scalar(out=zr[:], in0=zn[:], scalar1=0.0,
                                            scalar2=None, op0=ALU.max)

                nc.vector.tensor_tensor(out=hsb[0][:, b * HID:(b + 1) * HID],
                                        in0=zr[:], in1=res[:], op=ALU.add)
                write_h(0, b, hsb[0][:, b * HID:(b + 1) * HID])
                maybe_allgather(0, b)

            # =================== Layers 2,3 ===================
            for li, (wl, wr, bb) in enumerate(
                    [("Wl2", "Wr2", "b2"), ("Wl3", "Wr3", "b3")]):
                new_layer(h_src_aps(li))
                hprev = hsb[li % 2]
                hcur = hsb[(li + 1) % 2]
                for b in range(B):
                    hT_ps = ps.tile([HID, 128], BF16, tag="xT_ps", bufs=1)
                    nc.tensor.transpose(hT_ps[:], hprev[:, b * HID:(b + 1) * HID],
                                        ident_t[:])
                    hT = spool.tile([HID, 128], BF16, tag="hT")
                    nc.scalar.activation(hT[:], hT_ps[:], AF.Copy)
                    agg_ps = ps.tile([HID, 128], F32, tag="agg", bufs=2)
                    has_agg = scatter(b, agg_ps)
                    aggT = spool.tile([HID, 128], BF16, tag="aggT")
                    if has_agg:
                        nc.scalar.activation(aggT[:], agg_ps[:], AF.Copy)
                    else:
                        nc.vector.memset(aggT[:], 0.0)

                    zA = ps.tile([128, HID], F32, tag="zA", bufs=2)
                    nc.tensor.matmul(zA[:], lhsT=aggT[:], rhs=w_t[wl][:],
                                     start=True, stop=True)
                    zB = ps.tile([128, HID], F32, tag="zB", bufs=2)
                    nc.tensor.matmul(zB[:], lhsT=hT[:], rhs=w_t[wr][:],
                                     start=True, stop=False)
                    nc.tensor.matmul(zB[:], lhsT=ones_t[:], rhs=b_t[bb][:],
                                     start=False, stop=True)

                    sA = spool.tile([128, HID], F32, tag="sA")
                    nc.vector.tensor_scalar(
                        out=sA[:], in0=zA[:], scalar1=deginv_t[:, b:b + 1],
                        scalar2=None, op0=ALU.mult)
                    z = spool.tile([128, HID], F32, tag="z")
                    nc.vector.tensor_tensor(out=z[:], in0=sA[:], in1=zB[:],
                                            op=ALU.add)
                    nc.scalar.activation(hcur[:, b * HID:(b + 1) * HID],
                                         z[:], AF.Relu)
                    write_h(li + 1, b, hcur[:, b * HID:(b + 1) * HID])
                    maybe_allgather(li + 1, b)

            # =================== Layer 4 ===================
            new_layer(h_src_aps(2))
            hprev = hsb[0]
            for b in range(B):
                hT_ps = ps.tile([HID, 128], BF16, tag="xT_ps", bufs=1)
                nc.tensor.transpose(hT_ps[:], hprev[:, b * HID:(b + 1) * HID],
                                    ident_t[:])
                hT = spool.tile([HID, 128], BF16, tag="hT")
                nc.scalar.activation(hT[:], hT_ps[:], AF.Copy)
                agg_ps = ps.tile([HID, 128], F32, tag="agg", bufs=2)
                has_agg = scatter(b, agg_ps)
                aggT = spool.tile([HID, 128], BF16, tag="aggT")
                if has_agg:
                    nc.scalar.activation(aggT[:], agg_ps[:], AF.Copy)
                else:
                    nc.vector.memset(aggT[:], 0.0)

                oA = ps.tile([128, 1], F32, tag="zA", bufs=2)
                nc.tensor.matmul(oA[:], lhsT=aggT[:], rhs=w_t["Wl4"][:],
                                 start=True, stop=True)
                oB = ps.tile([128, 1], F32, tag="zB", bufs=2)
                nc.tensor.matmul(oB[:], lhsT=hT[:], rhs=w_t["Wr4"][:],
                                 start=True, stop=False)
                nc.tensor.matmul(oB[:], lhsT=ones_t[:], rhs=b_t["b4"][:],
                                 start=False, stop=True)
                t4 = spool.tile([128, 1], F32, tag="t4")
                nc.vector.tensor_scalar(
                    out=t4[:], in0=oA[:], scalar1=deginv_t[:, b:b + 1],
                    scalar2=None, op0=ALU.mult)
                nc.vector.tensor_tensor(out=out_sb[:, b:b + 1], in0=t4[:],
                                        in1=oB[:], op=ALU.add)

            nc.sync.dma_start(
                out=out_d[:].rearrange("(b p) -> p b", p=128), in_=out_sb[:])

    nc.compile()
    return nc


# ---------------------------------------------------------------------------
# Self-contained entry point


def _ensure_ntff_hook_package():
    import os
    site = "/root/.axon_site"
    try:
        pkg = os.path.join(site, "antenv")
        os.makedirs(pkg, exist_ok=True)
        init = os.path.join(pkg, "__init__.py")
        if not os.path.exists(init):
            with open(init, "w") as f:
                f.write("import pkgutil\n__path__ = pkgutil.extend_path(__path__, __name__)\n")
        hooks = os.path.join(pkg, "axon_hooks.py")
        if not os.path.exists(hooks):
            with open(hooks, "w") as f:
                f.write(
                    "_H = None\n"
                    "def set_axon_ntff_profile_hook(h):\n"
                    "    global _H\n"
                    "    _H = h\n"
                    "def get_axon_ntff_profile_hook():\n"
                    "    return _H\n")
    except Exception:
        pass


_ensure_ntff_hook_package()

_CACHE = {}
LAST_EXEC_NS = None


def _run(inputs, trace=True):
    x = np.asarray(inputs["x"], np.float32)
    edge_index = np.asarray(inputs["edge_index"])
    cfg = Cfg(x.shape[0])
    weights = {k: v for k, v in inputs.items() if k not in ("x", "edge_index")}
    in_maps, layout, ln_identity = preprocess(cfg, x, edge_index, weights)

    key = (x.shape, edge_index.shape, layout["NT"], layout["COLS"], ln_identity,
           SPLIT_AG, WT, GCAP)
    if key in _CACHE:
        nc = _CACHE[key]
    else:
        nc = build_program(cfg, layout, ln_identity)
        _CACHE[key] = nc

    from concourse.bass_utils import run_bass_kernel_spmd
    import concourse.bass_utils as bu
    bu.upload_artifacts = lambda d: d
    res = run_bass_kernel_spmd(nc, in_maps, core_ids=list(range(NCORES)),
                               trace=trace)
    outs = [res.results[c]["out"] for c in range(NCORES)]
    out = np.concatenate([np.asarray(o)[:cfg.NPC] for o in outs])
    return out.astype(np.float32), res.exec_time_ns


def kernel(**inputs):
    global LAST_EXEC_NS
    try:
        out, ns = _run(inputs, trace=True)
        LAST_EXEC_NS = ns
        return out
    except Exception:
        out, _ = _run(inputs, trace=False)
        LAST_EXEC_NS = None
        return out


if __name__ == "__main__":
    d = np.load('/tmp/ref_data.npz')
    inputs = {k[3:]: d[k] for k in d.files if k.startswith('in_')}
    cfg = Cfg(np.asarray(inputs['x']).shape[0])
    weights = {k: v for k, v in inputs.items() if k not in ('x', 'edge_index')}
    in_maps, layout, lnid = preprocess(
        cfg, np.asarray(inputs['x'], np.float32),
        np.asarray(inputs['edge_index']), weights)
    print("NTq:", layout["NTq"], "NT:", layout["NT"], "SLOTS:", layout["SLOTS"])
    print("COLS:", layout["COLS"], "MAXSPAN:", layout["MAXSPAN"],
          "mean cols_b:", np.mean(layout["cols_b"]))
    print("calls:", len(layout["calls"]))
    E = np.asarray(inputs['edge_index']).shape[1]
    print("slots/core vs E/8:", layout["SLOTS"], E / 8,
          "pad frac:", 1 - E / 8 / layout["SLOTS"])



# revision 28
# speedup vs baseline: 1.1267x; 1.1267x over previous
"""Self-contained Trainium2 Bass kernel for the 4-layer GraphSAGE GNN
(nn_EnhancedClassifier): kernel(**inputs) -> np.ndarray [100000] f32.

Runs SPMD on 8 NeuronCores via run_bass_kernel_spmd.

v2 strategy: dst-partition nodes across 8 cores. Per core, edges are
sorted by (src-quarter stream, dst_block) and packed into 4 per-chunk
tile streams with unit-anchored scheduling: groups of UNIT dst blocks
share a start offset = cumulative max edge count over cores (keeps the
SPMD program uniform with only ~5% pad, vs 25% for per-block max), and
tiles at block boundaries are shared by adjacent blocks (the one-hot
zeroes foreign edges). x is zero-padded to 128 feats bf16 so all 4
layers gather 256B rows. h_full is split into 4 per-quarter Shared DRAM
tensors so each AllGather quarter unblocks that chunk's gathers early.
The own-path h stays resident in SBUF between layers; PSUM->SBUF copies
run on the Scalar engine to keep DVE free for one-hot builds.
"""
import sys
sys.path.insert(0, '/opt/trn_rl_repo')
import numpy as np
import ml_dtypes
from concourse import bass, bacc, mybir, tile

BF16 = mybir.dt.bfloat16
F32 = mybir.dt.float32
FP8 = mybir.dt.float8e4
AF = mybir.ActivationFunctionType
ALU = mybir.AluOpType

NCORES = 8

# --- Patch Tile's DMASW lane assignment to be SWDGE-queue-aware: lane%4 must
# equal the instruction's queue_num or the runtime rejects the sem update.
import concourse.tile_sem_assignment as _tsa
from concourse import bass_isa as _bisa

if not getattr(_tsa, "_gnn_queue_patch", False):
    _orig_assign_tick = _tsa.TileClockTick._assign_tick

    def _assign_tick_qaware(self, inst):
        if isinstance(inst, mybir.InstDMAGatherAnt):
            q = inst.queue_num
            rot = self.__dict__.setdefault("_gnn_qrot", {})
            k = rot.get(q, 0)
            rot[q] = k ^ 1
            self.next_sw_dma_idx = q + 4 * k
        elif (isinstance(inst, _tsa.DMAInst)
              and inst.engine == mybir.EngineType.Pool
              and not isinstance(inst, _bisa.UserSyncedRemoteDMADescs)):
            rot = self.__dict__.setdefault("_gnn_qrot", {})
            k = rot.get(0, 0)
            rot[0] = k ^ 1
            self.next_sw_dma_idx = 4 * k
        return _orig_assign_tick(self, inst)

    _tsa.TileClockTick._assign_tick = _assign_tick_qaware
    _tsa._gnn_queue_patch = True

IN_F = 64
HID = 128
PAD_DSTLOC = 1000.0
NQ = 4              # chunk streams / AllGather quarters
import os as _os
WT = int(_os.environ.get("GNN_WT", "8"))   # tiles per dma_gather call
SPLIT_AG = int(_os.environ.get("GNN_SPLIT_AG", "1"))   # per-quarter AllGathers
# one-hot micro-block width: edges are dst-sorted into W-wide micro blocks;
# each one-hot matmul is [128 slots x W dsts], so one-hot DMA bytes scale
# with W (W=64 halves them vs 128). 128/W micro blocks share one PSUM tile.
OHW = int(_os.environ.get("GNN_OHW", "64"))
UNIT = 128 // OHW   # micro blocks per anchored unit (= per macro block)
# one-hot DMA batching: micro blocks are grouped until their span-columns
# reach GCAP (bytes/partition = GCAP*OHW), fetched with ONE dma_start.
GCAP = int(_os.environ.get("GNN_GCAP", str(12288 // OHW)))


class Cfg:
    def __init__(self, n_nodes):
        self.N = n_nodes
        self.NPC = n_nodes // NCORES
        assert self.NPC * NCORES == self.N
        self.B = (self.NPC + 127) // 128             # blocks per core
        self.ROWS = self.B * 128                     # padded rows per core
        self.GROWS = self.ROWS * NCORES
        # block-aligned quarters of each core's rows
        bq = self.B // NQ
        extra = self.B - bq * NQ
        self.qblocks = [bq + (1 if k < extra else 0) for k in range(NQ)]
        self.qrows = [q * 128 for q in self.qblocks]
        self.qstart = np.concatenate([[0], np.cumsum(self.qrows)]).astype(np.int64)
        self.chunk_rows = [NCORES * r for r in self.qrows]
        assert max(self.chunk_rows) <= 32768, "idx must fit int16"


def preprocess(cfg, x, edge_index, weights):
    src = edge_index[0].astype(np.int64)
    dst = edge_index[1].astype(np.int64)
    N, B = cfg.N, cfg.B

    deg = np.bincount(dst, minlength=N).astype(np.float32)
    deginv = 1.0 / np.maximum(deg, 1.0)

    s_core = src // cfg.NPC
    s_loc = src % cfg.NPC
    s_q = np.searchsorted(cfg.qstart[1:], s_loc, side='right')
    qrows_a = np.asarray(cfg.qrows, np.int64)
    s_row = s_core * qrows_a[s_q] + (s_loc - cfg.qstart[s_q])   # chunk-relative

    d_core = dst // cfg.NPC
    d_loc = dst % cfg.NPC
    BM = B * UNIT                 # micro blocks (OHW dsts each) per core
    d_block = d_loc // OHW        # micro block index

    # within each (core, stream, micro-block) run, order slots by ascending
    # src row: the one-hot encodes slot->dst anyway, and ascending gather
    # addresses give the HBM better locality than dst-sorted (random) reads.
    order = np.lexsort((s_row, d_block, s_q, d_core))
    s_row_s = s_row[order]
    d_loc_s = d_loc[order]
    key = (d_core[order] * NQ + s_q[order]) * BM + d_block[order]
    gstart = np.searchsorted(key, np.arange(NCORES * NQ * BM + 1))

    # per (core, q, micro) counts
    cnt = (gstart[1:] - gstart[:-1]).reshape(NCORES, NQ, BM)

    # ---- unit-anchored stream scheduling (uniform across cores); one unit
    # = one macro block (128 dsts) = UNIT micro blocks sharing slot slack
    NU = B
    S = np.zeros((NQ, NU + 1), np.int64)            # unit start slots per stream
    ucnt_max = np.zeros((NQ, NU), np.int64)
    for q in range(NQ):
        for u in range(NU):
            b0, b1 = u * UNIT, min(BM, (u + 1) * UNIT)
            ucnt_max[q, u] = cnt[:, q, b0:b1].sum(axis=1).max()
        S[q, 1:] = np.cumsum(ucnt_max[q])
    NTq = [int((S[q, NU] + 127) // 128) for q in range(NQ)]
    qtile0 = np.concatenate([[0], np.cumsum(NTq)]).astype(np.int64)
    NT = int(qtile0[-1])
    SLOTS = NT * 128

    # per-core micro-block positions within streams
    pos0 = np.zeros((NCORES, NQ, BM), np.int64)
    pos1 = np.zeros((NCORES, NQ, BM), np.int64)
    for q in range(NQ):
        for u in range(NU):
            b0, b1 = u * UNIT, min(BM, (u + 1) * UNIT)
            run = np.cumsum(
                np.concatenate([np.zeros((NCORES, 1), np.int64),
                                cnt[:, q, b0:b1]], axis=1), axis=1)
            pos0[:, q, b0:b1] = S[q, u] + run[:, :-1]
            pos1[:, q, b0:b1] = S[q, u] + run[:, 1:]

    # union spans per (micro, q) across cores
    uspans = []
    for m in range(BM):
        sp = []
        for q in range(NQ):
            mask = pos1[:, q, m] > pos0[:, q, m]
            if mask.any():
                t0 = int(pos0[mask, q, m].min() // 128)
                t1 = int(-(-pos1[mask, q, m].max() // 128))
                sp.append((q, t0, t1))
        uspans.append(sp)
    cols_b = [sum(t1 - t0 for (_, t0, t1) in uspans[m]) for m in range(BM)]
    COLS = int(sum(cols_b))
    col0_b = np.concatenate([[0], np.cumsum(cols_b)]).astype(np.int64)
    MAXSPAN = max(cols_b)

    # gather calls: interleave streams window-major
    calls = []
    for lo in range(0, max(NTq), WT):
        for q in range(NQ):
            if lo < NTq[q]:
                calls.append((q, lo, min(WT, NTq[q] - lo)))

    # one-hot load groups: consecutive micros packed until GCAP span-columns
    groups = []           # (m0, nmicros, col0, ncols)
    blk2grp = np.zeros(BM, np.int64)
    m = 0
    while m < BM:
        m0, ctot = m, 0
        while m < BM and (m == m0 or ctot + cols_b[m] <= GCAP):
            ctot += cols_b[m]
            blk2grp[m] = len(groups)
            m += 1
        groups.append((m0, m - m0, int(col0_b[m0]), ctot))
    assert max(g[3] for g in groups) <= max(GCAP, max(cols_b))

    layout = dict(NTq=NTq, qtile0=qtile0, NT=NT, SLOTS=SLOTS, uspans=uspans,
                  cols_b=cols_b, col0_b=col0_b, COLS=COLS, MAXSPAN=MAXSPAN,
                  calls=calls, groups=groups, blk2grp=blk2grp)

    # ---- weights / constants
    W = {k: np.asarray(v) for k, v in weights.items()}

    def padw(w):
        out = np.zeros((HID, HID), np.float32)
        out[:w.shape[0], :w.shape[1]] = w
        return out.astype(ml_dtypes.bfloat16)

    wcast = {
        "Wl1": padw(W["Wl1"]), "Wr1": padw(W["Wr1"]), "Wres": padw(W["Wres"]),
        "Wl2": W["Wl2"].astype(ml_dtypes.bfloat16),
        "Wr2": W["Wr2"].astype(ml_dtypes.bfloat16),
        "Wl3": W["Wl3"].astype(ml_dtypes.bfloat16),
        "Wr3": W["Wr3"].astype(ml_dtypes.bfloat16),
        "Wl4": W["Wl4"].astype(ml_dtypes.bfloat16),
        "Wr4": W["Wr4"].astype(ml_dtypes.bfloat16),
    }
    brow = {k: W[k].reshape(1, -1).astype(ml_dtypes.bfloat16)
            for k in ["b1", "bres", "b2", "b3", "b4"]}
    ln_g = W["ln_g"].astype(np.float32)
    ln_b = W["ln_b"].astype(np.float32)
    ln_identity = bool(np.all(ln_g == 1.0) and np.all(ln_b == 0.0))
    ln_g_rep = np.broadcast_to(ln_g, (128, HID)).copy()
    ln_b_rep = np.broadcast_to(ln_b, (128, HID)).copy()

    iota_bf = np.broadcast_to(np.arange(128, dtype=np.float32), (128, 128)).astype(
        ml_dtypes.bfloat16).copy()
    ones_row = np.ones((1, 128), ml_dtypes.bfloat16)
    ident_bf = np.eye(128, dtype=ml_dtypes.bfloat16)

    # x zero-padded to 128 feats bf16, laid out [chunk q][core][qrows]
    x_cast = np.asarray(x, np.float32).astype(ml_dtypes.bfloat16)
    xq = []
    for q in range(NQ):
        xb = np.zeros((cfg.chunk_rows[q], HID), ml_dtypes.bfloat16)
        for c in range(NCORES):
            lo = c * cfg.NPC + cfg.qstart[q]
            n = int(min(cfg.qrows[q], max(0, cfg.NPC - cfg.qstart[q])))
            if n > 0:
                xb[c * cfg.qrows[q]: c * cfg.qrows[q] + n, :IN_F] = x_cast[lo:lo + n]
        xq.append(xb)

    # ---- per-core inputs
    in_maps = []
    for c in range(NCORES):
        idx_lin = np.zeros(SLOTS, np.int16)
        for q in range(NQ):
            g0 = (c * NQ + q) * BM
            for mb in range(BM):
                lo_e, hi_e = gstart[g0 + mb], gstart[g0 + mb + 1]
                n = int(hi_e - lo_e)
                if n == 0:
                    continue
                s0 = int(pos0[c, q, mb])
                gslot = qtile0[q] * 128 + s0
                idx_lin[gslot:gslot + n] = s_row_s[lo_e:hi_e].astype(np.int16)
        # host-built one-hot, fp8: oh_all[p, col*OHW + j] = 1 iff the edge at
        # slot (tile(col), p) belongs to micro(col) and has dst j (local)
        oh_all = np.zeros((128, COLS * OHW), ml_dtypes.float8_e4m3fn)
        col = 0
        for mb in range(BM):
            for (q, t0, t1) in uspans[mb]:
                g0 = (c * NQ + q) * BM
                lo_e, hi_e = gstart[g0 + mb], gstart[g0 + mb + 1]
                n = int(hi_e - lo_e)
                if n:
                    s0 = int(pos0[c, q, mb])
                    dl = d_loc_s[lo_e:hi_e] - mb * OHW
                    sl = np.arange(s0, s0 + n)
                    tt = sl // 128 - t0
                    pp = sl % 128
                    oh_all[pp, (col + tt) * OHW + dl] = 1.0
                col += (t1 - t0)
        assert col == COLS

        idx_pk = idx_lin.reshape(SLOTS // 16, 16).T
        idx_pk = np.tile(idx_pk, (8, 1))

        dinv_col = np.ones((128, B), np.float32)
        basec = c * cfg.NPC
        for b in range(B):
            n_real = min(128, cfg.NPC - b * 128)
            dinv_col[:n_real, b] = deginv[basec + b * 128: basec + b * 128 + n_real]

        x_own = np.zeros((cfg.ROWS, HID), ml_dtypes.bfloat16)
        x_own[:cfg.NPC, :IN_F] = x_cast[c * cfg.NPC:(c + 1) * cfg.NPC]

        m = {
            "idx16": np.ascontiguousarray(idx_pk),
            "oh_all": oh_all,
            "deginv": dinv_col,
            "x_own": x_own,
            "ones_row": ones_row,
            "ident": ident_bf,
            "ln_g_rep": ln_g_rep,
            "ln_b_rep": ln_b_rep,
        }
        for q in range(NQ):
            m[f"x_q{q}"] = xq[q]
        m.update(wcast)
        m.update(brow)
        in_maps.append(m)

    return in_maps, layout, ln_identity


def build_program(cfg, layout, ln_identity):
    B, ROWS = cfg.B, cfg.ROWS
    COLS, SLOTS = layout["COLS"], layout["SLOTS"]
    uspans, col0_b = layout["uspans"], layout["col0_b"]
    qtile0, calls = layout["qtile0"], layout["calls"]
    MAXSPAN = layout["MAXSPAN"]
    nc = bacc.Bacc("TRN2", target_bir_lowering=False, debug=False,
                   num_devices=NCORES, num_swdge_queues=4)

    x_q = [nc.dram_tensor(f"x_q{q}", [cfg.chunk_rows[q], HID], BF16,
                          kind="ExternalInput") for q in range(NQ)]
    x_own = nc.dram_tensor("x_own", [ROWS, HID], BF16, kind="ExternalInput")
    idx_d = nc.dram_tensor("idx16", [128, SLOTS // 16], mybir.dt.int16,
                           kind="ExternalInput")
    oh_d = nc.dram_tensor("oh_all", [128, COLS * OHW], FP8, kind="ExternalInput")
    deginv_d = nc.dram_tensor("deginv", [128, B], F32, kind="ExternalInput")
    ones_d = nc.dram_tensor("ones_row", [1, 128], BF16, kind="ExternalInput")
    ident_d = nc.dram_tensor("ident", [128, 128], BF16, kind="ExternalInput")
    lng_d = nc.dram_tensor("ln_g_rep", [128, HID], F32, kind="ExternalInput")
    lnb_d = nc.dram_tensor("ln_b_rep", [128, HID], F32, kind="ExternalInput")
    wd = {k: nc.dram_tensor(k, [HID, HID], BF16, kind="ExternalInput")
          for k in ["Wl1", "Wr1", "Wres", "Wl2", "Wr2", "Wl3", "Wr3"]}
    wd["Wl4"] = nc.dram_tensor("Wl4", [HID, 1], BF16, kind="ExternalInput")
    wd["Wr4"] = nc.dram_tensor("Wr4", [HID, 1], BF16, kind="ExternalInput")
    bd = {k: nc.dram_tensor(k, [1, HID], BF16, kind="ExternalInput")
          for k in ["b1", "bres", "b2", "b3"]}
    bd["b4"] = nc.dram_tensor("b4", [1, 1], BF16, kind="ExternalInput")

    out_d = nc.dram_tensor("out", [ROWS], F32, kind="ExternalOutput")
    rg = [list(range(NCORES))]

    with tile.TileContext(nc) as tc:
        with (
            tc.tile_pool(name="dramp", bufs=1, space="DRAM") as dramp,
            tc.tile_pool(name="const", bufs=1) as constp,
            tc.tile_pool(name="meta", bufs=1) as metap,
            tc.tile_pool(name="gpool", bufs=10) as gpool,
            tc.tile_pool(name="ohpool", bufs=3) as ohpool,
            tc.tile_pool(name="spool", bufs=4) as spool,
            tc.tile_pool(name="hres", bufs=1) as hresp,
            tc.tile_pool(name="outp", bufs=1) as outp,
            tc.tile_pool(name="ps", bufs=2, space="PSUM") as ps,
        ):
            if SPLIT_AG:
                # per-quarter local h and Shared gathered h (3 layers x NQ)
                h_own = [[dramp.tile([cfg.qrows[k], HID], BF16,
                                     tag=f"h_own{l}_{k}", name=f"h_own{l}_{k}")
                          for k in range(NQ)] for l in range(3)]
                h_full = [[dramp.tile([cfg.chunk_rows[k], HID], BF16,
                                      tag=f"h_full{l}_{k}", name=f"h_full{l}_{k}",
                                      addr_space="Shared") for k in range(NQ)]
                          for l in range(3)]
            else:
                h_own1 = [dramp.tile([ROWS, HID], BF16, tag=f"h_own{l}",
                                     name=f"h_own{l}") for l in range(3)]
                h_full1 = [dramp.tile([cfg.GROWS, HID], BF16, tag=f"h_full{l}",
                                      name=f"h_full{l}", addr_space="Shared")
                           for l in range(3)]
            coff = np.concatenate(
                [[0], np.cumsum(cfg.chunk_rows)]).astype(np.int64)

            def h_src_aps(l):
                if SPLIT_AG:
                    return [h_full[l][k][:] for k in range(NQ)]
                return [h_full1[l][int(coff[k]):int(coff[k + 1]), :]
                        for k in range(NQ)]

            idx_t = metap.tile([128, SLOTS // 16], mybir.dt.int16)
            nc.sync.dma_start(out=idx_t[:], in_=idx_d[:])
            deginv_t = metap.tile([128, B], F32)
            nc.sync.dma_start(out=deginv_t[:], in_=deginv_d[:])
            ones_t = constp.tile([1, 128], BF16)
            nc.sync.dma_start(out=ones_t[:], in_=ones_d[:])
            ident_t = constp.tile([128, 128], BF16)
            nc.sync.dma_start(out=ident_t[:], in_=ident_d[:])
            eps_t = constp.tile([128, 1], F32)
            nc.vector.memset(eps_t[:], 1e-5)
            lng_t = constp.tile([128, HID], F32)
            nc.sync.dma_start(out=lng_t[:], in_=lng_d[:])
            lnb_t = constp.tile([128, HID], F32)
            nc.sync.dma_start(out=lnb_t[:], in_=lnb_d[:])
            w_t = {}
            for k, h in wd.items():
                w_t[k] = constp.tile(list(h.shape), BF16, tag=f"w_{k}", name=f"w_{k}")
                nc.sync.dma_start(out=w_t[k][:], in_=h[:])
            b_t = {}
            for k, h in bd.items():
                b_t[k] = constp.tile(list(h.shape), BF16, tag=f"b_{k}", name=f"b_{k}")
                nc.sync.dma_start(out=b_t[k][:], in_=h[:])

            out_sb = outp.tile([128, B], F32)
            hsb = [hresp.tile([128, B * HID], BF16, tag=f"hsb{i}",
                              name=f"hsb{i}") for i in range(2)]

            state = {"layer": 0}

            def new_layer(src_list):
                state["src"] = src_list
                state["G"] = {}
                state["nxt"] = 0
                state["cov"] = [0] * NQ
                state["layer"] += 1

            def issue_calls_until(need_q, need_t1):
                while state["cov"][need_q] < need_t1:
                    k = state["nxt"]
                    assert k < len(calls), (need_q, need_t1, state["cov"])
                    q, lo, nt = calls[k]
                    G = gpool.tile([128, WT * HID], BF16, tag="G",
                                   name=f"G_{state['layer']}_{k}")
                    base16 = (int(qtile0[q]) + lo) * 8
                    rows = nt * 128
                    nc.gpsimd.dma_gather(
                        out_ap=G[:, :nt * HID].rearrange(
                            "p (t e) -> p t e", e=HID),
                        in_ap=state["src"][q],
                        idxs_ap=idx_t[:, base16:base16 + rows // 16],
                        num_idxs=rows,
                        num_idxs_reg=rows,
                        elem_size=HID,
                        queue_num=k % 4,
                    )
                    state["G"][(q, lo // WT)] = G
                    state["cov"][q] = lo + nt
                    state["nxt"] = k + 1

            groups, blk2grp = layout["groups"], layout["blk2grp"]
            GMAX = max(g[3] for g in groups)

            def build_onehot(mb):
                # batched one-hot fetch: one dma_start per micro-block group
                nb = layout["cols_b"][mb]
                g = int(blk2grp[mb])
                m0, nbk, c0, ncols = groups[g]
                if mb == m0 and ncols > 0:
                    oh = ohpool.tile([128, GMAX * OHW], FP8, tag="oh")
                    nc.sync.dma_start(out=oh[:, :ncols * OHW],
                                      in_=oh_d[:, c0 * OHW:(c0 + ncols) * OHW])
                    state["ohg"] = oh
                if nb == 0:
                    return None, 0
                return state["ohg"], int(col0_b[mb]) - c0

            def scatter(b, agg_psum):
                # aggregate macro block b = UNIT micro blocks of OHW dsts,
                # each accumulating into its own PSUM column slice
                any_mm = False
                for j in range(UNIT):
                    mb = b * UNIT + j
                    for (q, t0, t1) in uspans[mb]:
                        issue_calls_until(q, t1)
                    oh, coff = build_onehot(mb)
                    sub = agg_psum[:, j * OHW:(j + 1) * OHW]
                    if oh is None:
                        nc.vector.memset(sub, 0.0)
                        continue
                    any_mm = True
                    n_mm = sum(t1 - t0 for (_, t0, t1) in uspans[mb])
                    col = coff
                    for (q, t0, t1) in uspans[mb]:
                        for t in range(t0, t1):
                            G = state["G"][(q, t // WT)]
                            off = (t % WT) * HID
                            nc.tensor.matmul(
                                sub, lhsT=G[:, off:off + HID],
                                rhs=oh[:, col * OHW:(col + 1) * OHW],
                                start=(col == coff),
                                stop=(col == coff + n_mm - 1))
                            col += 1
                return any_mm

            qlast = np.cumsum(cfg.qblocks) - 1      # last block of each quarter

            def maybe_allgather(l, b):
                if SPLIT_AG:
                    for k in range(NQ):
                        if b == qlast[k]:
                            nc.gpsimd.collective_compute(
                                "AllGather", ALU.bypass, replica_groups=rg,
                                ins=[h_own[l][k][:]], outs=[h_full[l][k][:]])
                elif b == B - 1:
                    nc.gpsimd.collective_compute(
                        "AllGather", ALU.bypass, replica_groups=rg,
                        ins=[h_own1[l][:]], outs=[h_full1[l][:]])

            def write_h(l, b, src_ap):
                if SPLIT_AG:
                    k = int(np.searchsorted(qlast, b))
                    b0 = int(qlast[k]) - cfg.qblocks[k] + 1
                    ro = (b - b0) * 128
                    nc.sync.dma_start(out=h_own[l][k][ro:ro + 128, :], in_=src_ap)
                else:
                    nc.sync.dma_start(
                        out=h_own1[l][b * 128:(b + 1) * 128, :], in_=src_ap)

            # =================== Layer 1 ===================
            new_layer([x_q[q][:] for q in range(NQ)])
            for b in range(B):
                xblk = spool.tile([128, HID], BF16, tag="hblk")
                nc.sync.dma_start(out=xblk[:], in_=x_own[b * 128:(b + 1) * 128, :])
                xT_ps = ps.tile([HID, 128], BF16, tag="xT_ps", bufs=1)
                nc.tensor.transpose(xT_ps[:], xblk[:], ident_t[:])
                xT = spool.tile([HID, 128], BF16, tag="hT")
                nc.scalar.activation(xT[:], xT_ps[:], AF.Copy)

                agg_ps = ps.tile([HID, 128], F32, tag="agg", bufs=2)
                has_agg = scatter(b, agg_ps)
                aggT = spool.tile([HID, 128], BF16, tag="aggT")
                if has_agg:
                    nc.scalar.activation(aggT[:], agg_ps[:], AF.Copy)
                else:
                    nc.vector.memset(aggT[:], 0.0)

                zA = ps.tile([128, HID], F32, tag="zA", bufs=2)
                nc.tensor.matmul(zA[:], lhsT=aggT[:], rhs=w_t["Wl1"][:],
                                 start=True, stop=True)
                zB = ps.tile([128, HID], F32, tag="zB", bufs=2)
                nc.tensor.matmul(zB[:], lhsT=xT[:], rhs=w_t["Wr1"][:],
                                 start=True, stop=False)
                nc.tensor.matmul(zB[:], lhsT=ones_t[:], rhs=b_t["b1"][:],
                                 start=False, stop=True)
                res = ps.tile([128, HID], F32, tag="res", bufs=1)
                nc.tensor.matmul(res[:], lhsT=xT[:], rhs=w_t["Wres"][:],
                                 start=True, stop=False)
                nc.tensor.matmul(res[:], lhsT=ones_t[:], rhs=b_t["bres"][:],
                                 start=False, stop=True)

                sA = spool.tile([128, HID], F32, tag="sA")
                nc.vector.tensor_scalar(
                    out=sA[:], in0=zA[:], scalar1=deginv_t[:, b:b + 1],
                    scalar2=None, op0=ALU.mult)
                z = spool.tile([128, HID], F32, tag="z")
                nc.vector.tensor_tensor(out=z[:], in0=sA[:], in1=zB[:], op=ALU.add)

                # LayerNorm via fused bn_stats: one DVE pass for mean+var,
                # then one Scalar pass Relu(z*rstd - mu*rstd)
                st6 = spool.tile([128, 6], F32, tag="st6")
                nc.vector.bn_stats(st6[:], z[:])
                agr = spool.tile([128, 2], F32, tag="agr")
                nc.vector.bn_aggr(agr[:], st6[:])
                std = spool.tile([128, 1], F32, tag="std")
                nc.scalar.activation(std[:], agr[:, 1:2], AF.Sqrt, bias=eps_t[:])
                rstd = spool.tile([128, 1], F32, tag="rstd")
                nc.vector.reciprocal(rstd[:], std[:])
                nmu = spool.tile([128, 1], F32, tag="nmu")
                nc.vector.tensor_scalar(out=nmu[:], in0=agr[:, 0:1],
                                        scalar1=rstd[:], scalar2=-1.0,
                                        op0=ALU.mult, op1=ALU.mult)

                if ln_identity:
                    zr = spool.tile([128, HID], F32, tag="zr")
                    nc.scalar.activation(zr[:], z[:], AF.Relu, bias=nmu[:],
                                         scale=rstd[:])
                else:
                    zn = spool.tile([128, HID], F32, tag="zn")
                    nc.scalar.activation(zn[:], z[:], AF.Identity, bias=nmu[:],
                                         scale=rstd[:])
                    nc.vector.tensor_tensor(out=zn[:], in0=zn[:], in1=lng_t[:],
                                            op=ALU.mult)
                    nc.vector.tensor_tensor(out=zn[:], in0=zn[:], in1=lnb_t[:],
                                            op=ALU.add)
                    zr = spool.tile([128, HID], F32, tag="zr")
                    nc.vector.tensor_scalar(out=zr[:], in0=zn[:], scalar1=0.0,
                                            scalar2=None, op0=ALU.max)

                nc.vector.tensor_tensor(out=hsb[0][:, b * HID:(b + 1) * HID],
                                        in0=zr[:], in1=res[:], op=ALU.add)
                write_h(0, b, hsb[0][:, b * HID:(b + 1) * HID])
                maybe_allgather(0, b)

            # =================== Layers 2,3 ===================
            for li, (wl, wr, bb) in enumerate(
                    [("Wl2", "Wr2", "b2"), ("Wl3", "Wr3", "b3")]):
                new_layer(h_src_aps(li))
                hprev = hsb[li % 2]
                hcur = hsb[(li + 1) % 2]
                for b in range(B):
                    hT_ps = ps.tile([HID, 128], BF16, tag="xT_ps", bufs=1)
                    nc.tensor.transpose(hT_ps[:], hprev[:, b * HID:(b + 1) * HID],
                                        ident_t[:])
                    hT = spool.tile([HID, 128], BF16, tag="hT")
                    nc.scalar.activation(hT[:], hT_ps[:], AF.Copy)
                    agg_ps = ps.tile([HID, 128], F32, tag="agg", bufs=2)
                    has_agg = scatter(b, agg_ps)
                    aggT = spool.tile([HID, 128], BF16, tag="aggT")
                    if has_agg:
                        nc.scalar.activation(aggT[:], agg_ps[:], AF.Copy)
                    else:
                        nc.vector.memset(aggT[:], 0.0)

                    zA = ps.tile([128, HID], F32, tag="zA", bufs=2)
                    nc.tensor.matmul(zA[:], lhsT=aggT[:], rhs=w_t[wl][:],
                                     start=True, stop=True)
                    zB = ps.tile([128, HID], F32, tag="zB", bufs=2)
                    nc.tensor.matmul(zB[:], lhsT=hT[:], rhs=w_t[wr][:],
                                     start=True, stop=False)
                    nc.tensor.matmul(zB[:], lhsT=ones_t[:], rhs=b_t[bb][:],
                                     start=False, stop=True)

                    sA = spool.tile([128, HID], F32, tag="sA")
                    nc.vector.tensor_scalar(
                        out=sA[:], in0=zA[:], scalar1=deginv_t[:, b:b + 1],
                        scalar2=None, op0=ALU.mult)
                    z = spool.tile([128, HID], F32, tag="z")
                    nc.vector.tensor_tensor(out=z[:], in0=sA[:], in1=zB[:],
                                            op=ALU.add)
                    nc.scalar.activation(hcur[:, b * HID:(b + 1) * HID],
                                         z[:], AF.Relu)
                    write_h(li + 1, b, hcur[:, b * HID:(b + 1) * HID])
                    maybe_allgather(li + 1, b)

            # =================== Layer 4 ===================
            new_layer(h_src_aps(2))
            hprev = hsb[0]
            for b in range(B):
                hT_ps = ps.tile([HID, 128], BF16, tag="xT_ps", bufs=1)
                nc.tensor.transpose(hT_ps[:], hprev[:, b * HID:(b + 1) * HID],
                                    ident_t[:])
                hT = spool.tile([HID, 128], BF16, tag="hT")
                nc.scalar.activation(hT[:], hT_ps[:], AF.Copy)
                agg_ps = ps.tile([HID, 128], F32, tag="agg", bufs=2)
                has_agg = scatter(b, agg_ps)
                aggT = spool.tile([HID, 128], BF16, tag="aggT")
                if has_agg:
                    nc.scalar.activation(aggT[:], agg_ps[:], AF.Copy)
                else:
                    nc.vector.memset(aggT[:], 0.0)

                oA = ps.tile([128, 1], F32, tag="zA", bufs=2)
                nc.tensor.matmul(oA[:], lhsT=aggT[:], rhs=w_t["Wl4"][:],
                                 start=True, stop=True)
                oB = ps.tile([128, 1], F32, tag="zB", bufs=2)
                nc.tensor.matmul(oB[:], lhsT=hT[:], rhs=w_t["Wr4"][:],
                                 start=True, stop=False)
                nc.tensor.matmul(oB[:], lhsT=ones_t[:], rhs=b_t["b4"][:],
                                 start=False, stop=True)
                t4 = spool.tile([128, 1], F32, tag="t4")
                nc.vector.tensor_scalar(
                    out=t4[:], in0=oA[:], scalar1=deginv_t[:, b:b + 1],
                    scalar2=None, op0=ALU.mult)
                nc.vector.tensor_tensor(out=out_sb[:, b:b + 1], in0=t4[:],
                                        in1=oB[:], op=ALU.add)

            nc.sync.dma_start(
                out=out_d[:].rearrange("(b p) -> p b", p=128), in_=out_sb[:])

    nc.compile()
    return nc


# ---------------------------------------------------------------------------
# Self-contained entry point


def _ensure_ntff_hook_package():
    import os
    site = "/root/.axon_site"
    try:
        pkg = os.path.join(site, "antenv")
        os.makedirs(pkg, exist_ok=True)
        init = os.path.join(pkg, "__init__.py")
        if not os.path.exists(init):
            with open(init, "w") as f:
                f.write("import pkgutil\n__path__ = pkgutil.extend_path(__path__, __name__)\n")
        hooks = os.path.join(pkg, "axon_hooks.py")
        if not os.path.exists(hooks):
            with open(hooks, "w") as f:
                f.write(
                    "_H = None\n"
                    "def set_axon_ntff_profile_hook(h):\n"
                    "    global _H\n"
                    "    _H = h\n"
                    "def get_axon_ntff_profile_hook():\n"
                    "    return _H\n")
    except Exception:
        pass


_ensure_ntff_hook_package()

_CACHE = {}
LAST_EXEC_NS = None


def _run(inputs, trace=True):
    x = np.asarray(inputs["x"], np.float32)
    edge_index = np.asarray(inputs["edge_index"])
    cfg = Cfg(x.shape[0])
    weights = {k: v for k, v in inputs.items() if k not in ("x", "edge_index")}
    in_maps, layout, ln_identity = preprocess(cfg, x, edge_index, weights)

    key = (x.shape, edge_index.shape, layout["NT"], layout["COLS"], ln_identity,
           SPLIT_AG, WT, GCAP, OHW)
    if key in _CACHE:
        nc = _CACHE[key]
    else:
        nc = build_program(cfg, layout, ln_identity)
        _CACHE[key] = nc

    from concourse.bass_utils import run_bass_kernel_spmd
    import concourse.bass_utils as bu
    bu.upload_artifacts = lambda d: d
    res = run_bass_kernel_spmd(nc, in_maps, core_ids=list(range(NCORES)),
                               trace=trace)
    outs = [res.results[c]["out"] for c in range(NCORES)]
    out = np.concatenate([np.asarray(o)[:cfg.NPC] for o in outs])
    return out.astype(np.float32), res.exec_time_ns


def kernel(**inputs):
    global LAST_EXEC_NS
    try:
        out, ns = _run(inputs, trace=True)
        LAST_EXEC_NS = ns
        return out
    except Exception:
        out, _ = _run(inputs, trace=False)
        LAST_EXEC_NS = None
        return out


if __name__ == "__main__":
    d = np.load('/tmp/ref_data.npz')
    inputs = {k[3:]: d[k] for k in d.files if k.startswith('in_')}
    cfg = Cfg(np.asarray(inputs['x']).shape[0])
    weights = {k: v for k, v in inputs.items() if k not in ('x', 'edge_index')}
    in_maps, layout, lnid = preprocess(
        cfg, np.asarray(inputs['x'], np.float32),
        np.asarray(inputs['edge_index']), weights)
    print("NTq:", layout["NTq"], "NT:", layout["NT"], "SLOTS:", layout["SLOTS"])
    print("COLS:", layout["COLS"], "MAXSPAN:", layout["MAXSPAN"],
          "mean cols_b:", np.mean(layout["cols_b"]))
    print("calls:", len(layout["calls"]))
    E = np.asarray(inputs['edge_index']).shape[1]
    print("slots/core vs E/8:", layout["SLOTS"], E / 8,
          "pad frac:", 1 - E / 8 / layout["SLOTS"])



# revision 31
# speedup vs baseline: 1.1450x; 1.0162x over previous
"""Self-contained Trainium2 Bass kernel for the 4-layer GraphSAGE GNN
(nn_EnhancedClassifier): kernel(**inputs) -> np.ndarray [100000] f32.

Runs SPMD on 8 NeuronCores via run_bass_kernel_spmd.

v2 strategy: dst-partition nodes across 8 cores. Per core, edges are
sorted by (src-quarter stream, dst_block) and packed into 4 per-chunk
tile streams with unit-anchored scheduling: groups of UNIT dst blocks
share a start offset = cumulative max edge count over cores (keeps the
SPMD program uniform with only ~5% pad, vs 25% for per-block max), and
tiles at block boundaries are shared by adjacent blocks (the one-hot
zeroes foreign edges). x is zero-padded to 128 feats bf16 so all 4
layers gather 256B rows. h_full is split into 4 per-quarter Shared DRAM
tensors so each AllGather quarter unblocks that chunk's gathers early.
The own-path h stays resident in SBUF between layers; PSUM->SBUF copies
run on the Scalar engine to keep DVE free for one-hot builds.
"""
import sys
sys.path.insert(0, '/opt/trn_rl_repo')
import numpy as np
import ml_dtypes
from concourse import bass, bacc, mybir, tile

BF16 = mybir.dt.bfloat16
F32 = mybir.dt.float32
FP8 = mybir.dt.float8e4
AF = mybir.ActivationFunctionType
ALU = mybir.AluOpType

NCORES = 8

# --- Patch Tile's DMASW lane assignment to be SWDGE-queue-aware: lane%4 must
# equal the instruction's queue_num or the runtime rejects the sem update.
import concourse.tile_sem_assignment as _tsa
from concourse import bass_isa as _bisa

if not getattr(_tsa, "_gnn_queue_patch", False):
    _orig_assign_tick = _tsa.TileClockTick._assign_tick

    def _assign_tick_qaware(self, inst):
        if isinstance(inst, mybir.InstDMAGatherAnt):
            q = inst.queue_num
            rot = self.__dict__.setdefault("_gnn_qrot", {})
            k = rot.get(q, 0)
            rot[q] = k ^ 1
            self.next_sw_dma_idx = q + 4 * k
        elif (isinstance(inst, _tsa.DMAInst)
              and inst.engine == mybir.EngineType.Pool
              and not isinstance(inst, _bisa.UserSyncedRemoteDMADescs)):
            rot = self.__dict__.setdefault("_gnn_qrot", {})
            k = rot.get(0, 0)
            rot[0] = k ^ 1
            self.next_sw_dma_idx = 4 * k
        return _orig_assign_tick(self, inst)

    _tsa.TileClockTick._assign_tick = _assign_tick_qaware
    _tsa._gnn_queue_patch = True

IN_F = 64
HID = 128
PAD_DSTLOC = 1000.0
NQ = 4              # chunk streams / AllGather quarters
import os as _os
WT = int(_os.environ.get("GNN_WT", "8"))   # tiles per dma_gather call
SPLIT_AG = int(_os.environ.get("GNN_SPLIT_AG", "1"))   # per-quarter AllGathers
# one-hot micro-block width: edges are dst-sorted into W-wide micro blocks;
# each one-hot matmul is [128 slots x W dsts], so one-hot DMA bytes scale
# with W (W=64 halves them vs 128). 128/W micro blocks share one PSUM tile.
OHW = int(_os.environ.get("GNN_OHW", "64"))
UNIT = 128 // OHW   # micro blocks per anchored unit (= per macro block)
# one-hot DMA batching: micro blocks are grouped until their span-columns
# reach GCAP (bytes/partition = GCAP*OHW), fetched with ONE dma_start.
GCAP = int(_os.environ.get("GNN_GCAP", str(12288 // OHW)))
# SWDGE descriptor-ring carveout (bytes/partition of SBUF). The default 16KB
# gives each (queue, DMA-engine) ring 64 descriptors -- LESS than one WT=8
# gather call (66), so calls serialize on ring space. 48KB lets ~3 calls per
# queue be in flight.
DMA_SCRATCH = int(_os.environ.get("GNN_DMA_SCRATCH", "49152"))


class Cfg:
    def __init__(self, n_nodes):
        self.N = n_nodes
        self.NPC = n_nodes // NCORES
        assert self.NPC * NCORES == self.N
        self.B = (self.NPC + 127) // 128             # blocks per core
        self.ROWS = self.B * 128                     # padded rows per core
        self.GROWS = self.ROWS * NCORES
        # block-aligned quarters of each core's rows
        bq = self.B // NQ
        extra = self.B - bq * NQ
        self.qblocks = [bq + (1 if k < extra else 0) for k in range(NQ)]
        self.qrows = [q * 128 for q in self.qblocks]
        self.qstart = np.concatenate([[0], np.cumsum(self.qrows)]).astype(np.int64)
        self.chunk_rows = [NCORES * r for r in self.qrows]
        assert max(self.chunk_rows) <= 32768, "idx must fit int16"


def preprocess(cfg, x, edge_index, weights):
    src = edge_index[0].astype(np.int64)
    dst = edge_index[1].astype(np.int64)
    N, B = cfg.N, cfg.B

    deg = np.bincount(dst, minlength=N).astype(np.float32)
    deginv = 1.0 / np.maximum(deg, 1.0)

    s_core = src // cfg.NPC
    s_loc = src % cfg.NPC
    s_q = np.searchsorted(cfg.qstart[1:], s_loc, side='right')
    qrows_a = np.asarray(cfg.qrows, np.int64)
    s_row = s_core * qrows_a[s_q] + (s_loc - cfg.qstart[s_q])   # chunk-relative

    d_core = dst // cfg.NPC
    d_loc = dst % cfg.NPC
    BM = B * UNIT                 # micro blocks (OHW dsts each) per core
    d_block = d_loc // OHW        # micro block index

    # within each (core, stream, micro-block) run, order slots by ascending
    # src row: the one-hot encodes slot->dst anyway, and ascending gather
    # addresses give the HBM better locality than dst-sorted (random) reads.
    order = np.lexsort((s_row, d_block, s_q, d_core))
    s_row_s = s_row[order]
    d_loc_s = d_loc[order]
    key = (d_core[order] * NQ + s_q[order]) * BM + d_block[order]
    gstart = np.searchsorted(key, np.arange(NCORES * NQ * BM + 1))

    # per (core, q, micro) counts
    cnt = (gstart[1:] - gstart[:-1]).reshape(NCORES, NQ, BM)

    # ---- unit-anchored stream scheduling (uniform across cores); one unit
    # = one macro block (128 dsts) = UNIT micro blocks sharing slot slack
    NU = B
    S = np.zeros((NQ, NU + 1), np.int64)            # unit start slots per stream
    ucnt_max = np.zeros((NQ, NU), np.int64)
    for q in range(NQ):
        for u in range(NU):
            b0, b1 = u * UNIT, min(BM, (u + 1) * UNIT)
            ucnt_max[q, u] = cnt[:, q, b0:b1].sum(axis=1).max()
        S[q, 1:] = np.cumsum(ucnt_max[q])
    NTq = [int((S[q, NU] + 127) // 128) for q in range(NQ)]
    qtile0 = np.concatenate([[0], np.cumsum(NTq)]).astype(np.int64)
    NT = int(qtile0[-1])
    SLOTS = NT * 128

    # per-core micro-block positions within streams
    pos0 = np.zeros((NCORES, NQ, BM), np.int64)
    pos1 = np.zeros((NCORES, NQ, BM), np.int64)
    for q in range(NQ):
        for u in range(NU):
            b0, b1 = u * UNIT, min(BM, (u + 1) * UNIT)
            run = np.cumsum(
                np.concatenate([np.zeros((NCORES, 1), np.int64),
                                cnt[:, q, b0:b1]], axis=1), axis=1)
            pos0[:, q, b0:b1] = S[q, u] + run[:, :-1]
            pos1[:, q, b0:b1] = S[q, u] + run[:, 1:]

    # union spans per (micro, q) across cores
    uspans = []
    for m in range(BM):
        sp = []
        for q in range(NQ):
            mask = pos1[:, q, m] > pos0[:, q, m]
            if mask.any():
                t0 = int(pos0[mask, q, m].min() // 128)
                t1 = int(-(-pos1[mask, q, m].max() // 128))
                sp.append((q, t0, t1))
        uspans.append(sp)
    cols_b = [sum(t1 - t0 for (_, t0, t1) in uspans[m]) for m in range(BM)]
    COLS = int(sum(cols_b))
    col0_b = np.concatenate([[0], np.cumsum(cols_b)]).astype(np.int64)
    MAXSPAN = max(cols_b)

    # gather calls: interleave streams window-major
    calls = []
    for lo in range(0, max(NTq), WT):
        for q in range(NQ):
            if lo < NTq[q]:
                calls.append((q, lo, min(WT, NTq[q] - lo)))

    # one-hot load groups: consecutive micros packed until GCAP span-columns
    groups = []           # (m0, nmicros, col0, ncols)
    blk2grp = np.zeros(BM, np.int64)
    m = 0
    while m < BM:
        m0, ctot = m, 0
        while m < BM and (m == m0 or ctot + cols_b[m] <= GCAP):
            ctot += cols_b[m]
            blk2grp[m] = len(groups)
            m += 1
        groups.append((m0, m - m0, int(col0_b[m0]), ctot))
    assert max(g[3] for g in groups) <= max(GCAP, max(cols_b))

    layout = dict(NTq=NTq, qtile0=qtile0, NT=NT, SLOTS=SLOTS, uspans=uspans,
                  cols_b=cols_b, col0_b=col0_b, COLS=COLS, MAXSPAN=MAXSPAN,
                  calls=calls, groups=groups, blk2grp=blk2grp)

    # ---- weights / constants
    W = {k: np.asarray(v) for k, v in weights.items()}

    def padw(w):
        out = np.zeros((HID, HID), np.float32)
        out[:w.shape[0], :w.shape[1]] = w
        return out.astype(ml_dtypes.bfloat16)

    wcast = {
        "Wl1": padw(W["Wl1"]), "Wr1": padw(W["Wr1"]), "Wres": padw(W["Wres"]),
        "Wl2": W["Wl2"].astype(ml_dtypes.bfloat16),
        "Wr2": W["Wr2"].astype(ml_dtypes.bfloat16),
        "Wl3": W["Wl3"].astype(ml_dtypes.bfloat16),
        "Wr3": W["Wr3"].astype(ml_dtypes.bfloat16),
        "Wl4": W["Wl4"].astype(ml_dtypes.bfloat16),
        "Wr4": W["Wr4"].astype(ml_dtypes.bfloat16),
    }
    brow = {k: W[k].reshape(1, -1).astype(ml_dtypes.bfloat16)
            for k in ["b1", "bres", "b2", "b3", "b4"]}
    ln_g = W["ln_g"].astype(np.float32)
    ln_b = W["ln_b"].astype(np.float32)
    ln_identity = bool(np.all(ln_g == 1.0) and np.all(ln_b == 0.0))
    ln_g_rep = np.broadcast_to(ln_g, (128, HID)).copy()
    ln_b_rep = np.broadcast_to(ln_b, (128, HID)).copy()

    iota_bf = np.broadcast_to(np.arange(128, dtype=np.float32), (128, 128)).astype(
        ml_dtypes.bfloat16).copy()
    ones_row = np.ones((1, 128), ml_dtypes.bfloat16)
    ident_bf = np.eye(128, dtype=ml_dtypes.bfloat16)

    # x zero-padded to 128 feats bf16, laid out [chunk q][core][qrows]
    x_cast = np.asarray(x, np.float32).astype(ml_dtypes.bfloat16)
    xq = []
    for q in range(NQ):
        xb = np.zeros((cfg.chunk_rows[q], HID), ml_dtypes.bfloat16)
        for c in range(NCORES):
            lo = c * cfg.NPC + cfg.qstart[q]
            n = int(min(cfg.qrows[q], max(0, cfg.NPC - cfg.qstart[q])))
            if n > 0:
                xb[c * cfg.qrows[q]: c * cfg.qrows[q] + n, :IN_F] = x_cast[lo:lo + n]
        xq.append(xb)

    # ---- per-core inputs
    in_maps = []
    for c in range(NCORES):
        idx_lin = np.zeros(SLOTS, np.int16)
        for q in range(NQ):
            g0 = (c * NQ + q) * BM
            for mb in range(BM):
                lo_e, hi_e = gstart[g0 + mb], gstart[g0 + mb + 1]
                n = int(hi_e - lo_e)
                if n == 0:
                    continue
                s0 = int(pos0[c, q, mb])
                gslot = qtile0[q] * 128 + s0
                idx_lin[gslot:gslot + n] = s_row_s[lo_e:hi_e].astype(np.int16)
        # host-built one-hot, fp8: oh_all[p, col*OHW + j] = 1 iff the edge at
        # slot (tile(col), p) belongs to micro(col) and has dst j (local)
        oh_all = np.zeros((128, COLS * OHW), ml_dtypes.float8_e4m3fn)
        col = 0
        for mb in range(BM):
            for (q, t0, t1) in uspans[mb]:
                g0 = (c * NQ + q) * BM
                lo_e, hi_e = gstart[g0 + mb], gstart[g0 + mb + 1]
                n = int(hi_e - lo_e)
                if n:
                    s0 = int(pos0[c, q, mb])
                    dl = d_loc_s[lo_e:hi_e] - mb * OHW
                    sl = np.arange(s0, s0 + n)
                    tt = sl // 128 - t0
                    pp = sl % 128
                    oh_all[pp, (col + tt) * OHW + dl] = 1.0
                col += (t1 - t0)
        assert col == COLS

        idx_pk = idx_lin.reshape(SLOTS // 16, 16).T
        idx_pk = np.tile(idx_pk, (8, 1))

        dinv_col = np.ones((128, B), np.float32)
        basec = c * cfg.NPC
        for b in range(B):
            n_real = min(128, cfg.NPC - b * 128)
            dinv_col[:n_real, b] = deginv[basec + b * 128: basec + b * 128 + n_real]

        x_own = np.zeros((cfg.ROWS, HID), ml_dtypes.bfloat16)
        x_own[:cfg.NPC, :IN_F] = x_cast[c * cfg.NPC:(c + 1) * cfg.NPC]

        m = {
            "idx16": np.ascontiguousarray(idx_pk),
            "oh_all": oh_all,
            "deginv": dinv_col,
            "x_own": x_own,
            "ones_row": ones_row,
            "ident": ident_bf,
            "ln_g_rep": ln_g_rep,
            "ln_b_rep": ln_b_rep,
        }
        for q in range(NQ):
            m[f"x_q{q}"] = xq[q]
        m.update(wcast)
        m.update(brow)
        in_maps.append(m)

    return in_maps, layout, ln_identity


def build_program(cfg, layout, ln_identity):
    B, ROWS = cfg.B, cfg.ROWS
    COLS, SLOTS = layout["COLS"], layout["SLOTS"]
    uspans, col0_b = layout["uspans"], layout["col0_b"]
    qtile0, calls = layout["qtile0"], layout["calls"]
    MAXSPAN = layout["MAXSPAN"]
    nc = bacc.Bacc("TRN2", target_bir_lowering=False, debug=False,
                   num_devices=NCORES, num_swdge_queues=4,
                   dynamic_dma_scratch_size=DMA_SCRATCH)

    x_q = [nc.dram_tensor(f"x_q{q}", [cfg.chunk_rows[q], HID], BF16,
                          kind="ExternalInput") for q in range(NQ)]
    x_own = nc.dram_tensor("x_own", [ROWS, HID], BF16, kind="ExternalInput")
    idx_d = nc.dram_tensor("idx16", [128, SLOTS // 16], mybir.dt.int16,
                           kind="ExternalInput")
    oh_d = nc.dram_tensor("oh_all", [128, COLS * OHW], FP8, kind="ExternalInput")
    deginv_d = nc.dram_tensor("deginv", [128, B], F32, kind="ExternalInput")
    ones_d = nc.dram_tensor("ones_row", [1, 128], BF16, kind="ExternalInput")
    ident_d = nc.dram_tensor("ident", [128, 128], BF16, kind="ExternalInput")
    lng_d = nc.dram_tensor("ln_g_rep", [128, HID], F32, kind="ExternalInput")
    lnb_d = nc.dram_tensor("ln_b_rep", [128, HID], F32, kind="ExternalInput")
    wd = {k: nc.dram_tensor(k, [HID, HID], BF16, kind="ExternalInput")
          for k in ["Wl1", "Wr1", "Wres", "Wl2", "Wr2", "Wl3", "Wr3"]}
    wd["Wl4"] = nc.dram_tensor("Wl4", [HID, 1], BF16, kind="ExternalInput")
    wd["Wr4"] = nc.dram_tensor("Wr4", [HID, 1], BF16, kind="ExternalInput")
    bd = {k: nc.dram_tensor(k, [1, HID], BF16, kind="ExternalInput")
          for k in ["b1", "bres", "b2", "b3"]}
    bd["b4"] = nc.dram_tensor("b4", [1, 1], BF16, kind="ExternalInput")

    out_d = nc.dram_tensor("out", [ROWS], F32, kind="ExternalOutput")
    rg = [list(range(NCORES))]

    with tile.TileContext(nc) as tc:
        with (
            tc.tile_pool(name="dramp", bufs=1, space="DRAM") as dramp,
            tc.tile_pool(name="const", bufs=1) as constp,
            tc.tile_pool(name="meta", bufs=1) as metap,
            tc.tile_pool(name="gpool", bufs=10) as gpool,
            tc.tile_pool(name="ohpool", bufs=3) as ohpool,
            tc.tile_pool(name="spool", bufs=4) as spool,
            tc.tile_pool(name="hres", bufs=1) as hresp,
            tc.tile_pool(name="outp", bufs=1) as outp,
            tc.tile_pool(name="ps", bufs=2, space="PSUM") as ps,
        ):
            if SPLIT_AG:
                # per-quarter local h and Shared gathered h (3 layers x NQ)
                h_own = [[dramp.tile([cfg.qrows[k], HID], BF16,
                                     tag=f"h_own{l}_{k}", name=f"h_own{l}_{k}")
                          for k in range(NQ)] for l in range(3)]
                h_full = [[dramp.tile([cfg.chunk_rows[k], HID], BF16,
                                      tag=f"h_full{l}_{k}", name=f"h_full{l}_{k}",
                                      addr_space="Shared") for k in range(NQ)]
                          for l in range(3)]
            else:
                h_own1 = [dramp.tile([ROWS, HID], BF16, tag=f"h_own{l}",
                                     name=f"h_own{l}") for l in range(3)]
                h_full1 = [dramp.tile([cfg.GROWS, HID], BF16, tag=f"h_full{l}",
                                      name=f"h_full{l}", addr_space="Shared")
                           for l in range(3)]
            coff = np.concatenate(
                [[0], np.cumsum(cfg.chunk_rows)]).astype(np.int64)

            def h_src_aps(l):
                if SPLIT_AG:
                    return [h_full[l][k][:] for k in range(NQ)]
                return [h_full1[l][int(coff[k]):int(coff[k + 1]), :]
                        for k in range(NQ)]

            idx_t = metap.tile([128, SLOTS // 16], mybir.dt.int16)
            nc.sync.dma_start(out=idx_t[:], in_=idx_d[:])
            deginv_t = metap.tile([128, B], F32)
            nc.sync.dma_start(out=deginv_t[:], in_=deginv_d[:])
            ones_t = constp.tile([1, 128], BF16)
            nc.sync.dma_start(out=ones_t[:], in_=ones_d[:])
            ident_t = constp.tile([128, 128], BF16)
            nc.sync.dma_start(out=ident_t[:], in_=ident_d[:])
            eps_t = constp.tile([128, 1], F32)
            nc.vector.memset(eps_t[:], 1e-5)
            lng_t = constp.tile([128, HID], F32)
            nc.sync.dma_start(out=lng_t[:], in_=lng_d[:])
            lnb_t = constp.tile([128, HID], F32)
            nc.sync.dma_start(out=lnb_t[:], in_=lnb_d[:])
            w_t = {}
            for k, h in wd.items():
                w_t[k] = constp.tile(list(h.shape), BF16, tag=f"w_{k}", name=f"w_{k}")
                nc.sync.dma_start(out=w_t[k][:], in_=h[:])
            b_t = {}
            for k, h in bd.items():
                b_t[k] = constp.tile(list(h.shape), BF16, tag=f"b_{k}", name=f"b_{k}")
                nc.sync.dma_start(out=b_t[k][:], in_=h[:])

            out_sb = outp.tile([128, B], F32)
            hsb = [hresp.tile([128, B * HID], BF16, tag=f"hsb{i}",
                              name=f"hsb{i}") for i in range(2)]

            state = {"layer": 0}

            def new_layer(src_list):
                state["src"] = src_list
                state["G"] = {}
                state["nxt"] = 0
                state["cov"] = [0] * NQ
                state["layer"] += 1

            def issue_calls_until(need_q, need_t1):
                while state["cov"][need_q] < need_t1:
                    k = state["nxt"]
                    assert k < len(calls), (need_q, need_t1, state["cov"])
                    q, lo, nt = calls[k]
                    G = gpool.tile([128, WT * HID], BF16, tag="G",
                                   name=f"G_{state['layer']}_{k}")
                    base16 = (int(qtile0[q]) + lo) * 8
                    rows = nt * 128
                    nc.gpsimd.dma_gather(
                        out_ap=G[:, :nt * HID].rearrange(
                            "p (t e) -> p t e", e=HID),
                        in_ap=state["src"][q],
                        idxs_ap=idx_t[:, base16:base16 + rows // 16],
                        num_idxs=rows,
                        num_idxs_reg=rows,
                        elem_size=HID,
                        queue_num=k % 4,
                    )
                    state["G"][(q, lo // WT)] = G
                    state["cov"][q] = lo + nt
                    state["nxt"] = k + 1

            groups, blk2grp = layout["groups"], layout["blk2grp"]
            GMAX = max(g[3] for g in groups)

            def build_onehot(mb):
                # batched one-hot fetch: one dma_start per micro-block group
                nb = layout["cols_b"][mb]
                g = int(blk2grp[mb])
                m0, nbk, c0, ncols = groups[g]
                if mb == m0 and ncols > 0:
                    oh = ohpool.tile([128, GMAX * OHW], FP8, tag="oh")
                    nc.sync.dma_start(out=oh[:, :ncols * OHW],
                                      in_=oh_d[:, c0 * OHW:(c0 + ncols) * OHW])
                    state["ohg"] = oh
                if nb == 0:
                    return None, 0
                return state["ohg"], int(col0_b[mb]) - c0

            def scatter(b, agg_psum):
                # aggregate macro block b = UNIT micro blocks of OHW dsts,
                # each accumulating into its own PSUM column slice
                any_mm = False
                for j in range(UNIT):
                    mb = b * UNIT + j
                    for (q, t0, t1) in uspans[mb]:
                        issue_calls_until(q, t1)
                    oh, coff = build_onehot(mb)
                    sub = agg_psum[:, j * OHW:(j + 1) * OHW]
                    if oh is None:
                        nc.vector.memset(sub, 0.0)
                        continue
                    any_mm = True
                    n_mm = sum(t1 - t0 for (_, t0, t1) in uspans[mb])
                    col = coff
                    for (q, t0, t1) in uspans[mb]:
                        for t in range(t0, t1):
                            G = state["G"][(q, t // WT)]
                            off = (t % WT) * HID
                            nc.tensor.matmul(
                                sub, lhsT=G[:, off:off + HID],
                                rhs=oh[:, col * OHW:(col + 1) * OHW],
                                start=(col == coff),
                                stop=(col == coff + n_mm - 1))
                            col += 1
                return any_mm

            qlast = np.cumsum(cfg.qblocks) - 1      # last block of each quarter

            def maybe_allgather(l, b):
                if SPLIT_AG:
                    for k in range(NQ):
                        if b == qlast[k]:
                            nc.gpsimd.collective_compute(
                                "AllGather", ALU.bypass, replica_groups=rg,
                                ins=[h_own[l][k][:]], outs=[h_full[l][k][:]])
                elif b == B - 1:
                    nc.gpsimd.collective_compute(
                        "AllGather", ALU.bypass, replica_groups=rg,
                        ins=[h_own1[l][:]], outs=[h_full1[l][:]])

            def write_h(l, b, src_ap):
                if SPLIT_AG:
                    k = int(np.searchsorted(qlast, b))
                    b0 = int(qlast[k]) - cfg.qblocks[k] + 1
                    ro = (b - b0) * 128
                    nc.sync.dma_start(out=h_own[l][k][ro:ro + 128, :], in_=src_ap)
                else:
                    nc.sync.dma_start(
                        out=h_own1[l][b * 128:(b + 1) * 128, :], in_=src_ap)

            # =================== Layer 1 ===================
            new_layer([x_q[q][:] for q in range(NQ)])
            for b in range(B):
                xblk = spool.tile([128, HID], BF16, tag="hblk")
                nc.sync.dma_start(out=xblk[:], in_=x_own[b * 128:(b + 1) * 128, :])
                xT_ps = ps.tile([HID, 128], BF16, tag="xT_ps", bufs=1)
                nc.tensor.transpose(xT_ps[:], xblk[:], ident_t[:])
                xT = spool.tile([HID, 128], BF16, tag="hT")
                nc.scalar.activation(xT[:], xT_ps[:], AF.Copy)

                agg_ps = ps.tile([HID, 128], F32, tag="agg", bufs=2)
                has_agg = scatter(b, agg_ps)
                aggT = spool.tile([HID, 128], BF16, tag="aggT")
                if has_agg:
                    nc.scalar.activation(aggT[:], agg_ps[:], AF.Copy)
                else:
                    nc.vector.memset(aggT[:], 0.0)

                zA = ps.tile([128, HID], F32, tag="zA", bufs=2)
                nc.tensor.matmul(zA[:], lhsT=aggT[:], rhs=w_t["Wl1"][:],
                                 start=True, stop=True)
                zB = ps.tile([128, HID], F32, tag="zB", bufs=2)
                nc.tensor.matmul(zB[:], lhsT=xT[:], rhs=w_t["Wr1"][:],
                                 start=True, stop=False)
                nc.tensor.matmul(zB[:], lhsT=ones_t[:], rhs=b_t["b1"][:],
                                 start=False, stop=True)
                res = ps.tile([128, HID], F32, tag="res", bufs=1)
                nc.tensor.matmul(res[:], lhsT=xT[:], rhs=w_t["Wres"][:],
                                 start=True, stop=False)
                nc.tensor.matmul(res[:], lhsT=ones_t[:], rhs=b_t["bres"][:],
                                 start=False, stop=True)

                sA = spool.tile([128, HID], F32, tag="sA")
                nc.vector.tensor_scalar(
                    out=sA[:], in0=zA[:], scalar1=deginv_t[:, b:b + 1],
                    scalar2=None, op0=ALU.mult)
                z = spool.tile([128, HID], F32, tag="z")
                nc.vector.tensor_tensor(out=z[:], in0=sA[:], in1=zB[:], op=ALU.add)

                # LayerNorm via fused bn_stats: one DVE pass for mean+var,
                # then one Scalar pass Relu(z*rstd - mu*rstd)
                st6 = spool.tile([128, 6], F32, tag="st6")
                nc.vector.bn_stats(st6[:], z[:])
                agr = spool.tile([128, 2], F32, tag="agr")
                nc.vector.bn_aggr(agr[:], st6[:])
                std = spool.tile([128, 1], F32, tag="std")
                nc.scalar.activation(std[:], agr[:, 1:2], AF.Sqrt, bias=eps_t[:])
                rstd = spool.tile([128, 1], F32, tag="rstd")
                nc.vector.reciprocal(rstd[:], std[:])
                nmu = spool.tile([128, 1], F32, tag="nmu")
                nc.vector.tensor_scalar(out=nmu[:], in0=agr[:, 0:1],
                                        scalar1=rstd[:], scalar2=-1.0,
                                        op0=ALU.mult, op1=ALU.mult)

                if ln_identity:
                    zr = spool.tile([128, HID], F32, tag="zr")
                    nc.scalar.activation(zr[:], z[:], AF.Relu, bias=nmu[:],
                                         scale=rstd[:])
                else:
                    zn = spool.tile([128, HID], F32, tag="zn")
                    nc.scalar.activation(zn[:], z[:], AF.Identity, bias=nmu[:],
                                         scale=rstd[:])
                    nc.vector.tensor_tensor(out=zn[:], in0=zn[:], in1=lng_t[:],
                                            op=ALU.mult)
                    nc.vector.tensor_tensor(out=zn[:], in0=zn[:], in1=lnb_t[:],
                                            op=ALU.add)
                    zr = spool.tile([128, HID], F32, tag="zr")
                    nc.vector.tensor_scalar(out=zr[:], in0=zn[:], scalar1=0.0,
                                            scalar2=None, op0=ALU.max)

                nc.vector.tensor_tensor(out=hsb[0][:, b * HID:(b + 1) * HID],
                                        in0=zr[:], in1=res[:], op=ALU.add)
                write_h(0, b, hsb[0][:, b * HID:(b + 1) * HID])
                maybe_allgather(0, b)

            # =================== Layers 2,3 ===================
            for li, (wl, wr, bb) in enumerate(
                    [("Wl2", "Wr2", "b2"), ("Wl3", "Wr3", "b3")]):
                new_layer(h_src_aps(li))
                hprev = hsb[li % 2]
                hcur = hsb[(li + 1) % 2]
                for b in range(B):
                    hT_ps = ps.tile([HID, 128], BF16, tag="xT_ps", bufs=1)
                    nc.tensor.transpose(hT_ps[:], hprev[:, b * HID:(b + 1) * HID],
                                        ident_t[:])
                    hT = spool.tile([HID, 128], BF16, tag="hT")
                    nc.scalar.activation(hT[:], hT_ps[:], AF.Copy)
                    agg_ps = ps.tile([HID, 128], F32, tag="agg", bufs=2)
                    has_agg = scatter(b, agg_ps)
                    aggT = spool.tile([HID, 128], BF16, tag="aggT")
                    if has_agg:
                        nc.scalar.activation(aggT[:], agg_ps[:], AF.Copy)
                    else:
                        nc.vector.memset(aggT[:], 0.0)

                    zA = ps.tile([128, HID], F32, tag="zA", bufs=2)
                    nc.tensor.matmul(zA[:], lhsT=aggT[:], rhs=w_t[wl][:],
                                     start=True, stop=True)
                    zB = ps.tile([128, HID], F32, tag="zB", bufs=2)
                    nc.tensor.matmul(zB[:], lhsT=hT[:], rhs=w_t[wr][:],
                                     start=True, stop=False)
                    nc.tensor.matmul(zB[:], lhsT=ones_t[:], rhs=b_t[bb][:],
                                     start=False, stop=True)

                    sA = spool.tile([128, HID], F32, tag="sA")
                    nc.vector.tensor_scalar(
                        out=sA[:], in0=zA[:], scalar1=deginv_t[:, b:b + 1],
                        scalar2=None, op0=ALU.mult)
                    z = spool.tile([128, HID], F32, tag="z")
                    nc.vector.tensor_tensor(out=z[:], in0=sA[:], in1=zB[:],
                                            op=ALU.add)
                    nc.scalar.activation(hcur[:, b * HID:(b + 1) * HID],
                                         z[:], AF.Relu)
                    write_h(li + 1, b, hcur[:, b * HID:(b + 1) * HID])
                    maybe_allgather(li + 1, b)

            # =================== Layer 4 ===================
            new_layer(h_src_aps(2))
            hprev = hsb[0]
            for b in range(B):
                hT_ps = ps.tile([HID, 128], BF16, tag="xT_ps", bufs=1)
                nc.tensor.transpose(hT_ps[:], hprev[:, b * HID:(b + 1) * HID],
                                    ident_t[:])
                hT = spool.tile([HID, 128], BF16, tag="hT")
                nc.scalar.activation(hT[:], hT_ps[:], AF.Copy)
                agg_ps = ps.tile([HID, 128], F32, tag="agg", bufs=2)
                has_agg = scatter(b, agg_ps)
                aggT = spool.tile([HID, 128], BF16, tag="aggT")
                if has_agg:
                    nc.scalar.activation(aggT[:], agg_ps[:], AF.Copy)
                else:
                    nc.vector.memset(aggT[:], 0.0)

                oA = ps.tile([128, 1], F32, tag="zA", bufs=2)
                nc.tensor.matmul(oA[:], lhsT=aggT[:], rhs=w_t["Wl4"][:],
                                 start=True, stop=True)
                oB = ps.tile([128, 1], F32, tag="zB", bufs=2)
                nc.tensor.matmul(oB[:], lhsT=hT[:], rhs=w_t["Wr4"][:],
                                 start=True, stop=False)
                nc.tensor.matmul(oB[:], lhsT=ones_t[:], rhs=b_t["b4"][:],
                                 start=False, stop=True)
                t4 = spool.tile([128, 1], F32, tag="t4")
                nc.vector.tensor_scalar(
                    out=t4[:], in0=oA[:], scalar1=deginv_t[:, b:b + 1],
                    scalar2=None, op0=ALU.mult)
                nc.vector.tensor_tensor(out=out_sb[:, b:b + 1], in0=t4[:],
                                        in1=oB[:], op=ALU.add)

            nc.sync.dma_start(
                out=out_d[:].rearrange("(b p) -> p b", p=128), in_=out_sb[:])

    nc.compile()
    return nc


# ---------------------------------------------------------------------------
# Self-contained entry point


def _ensure_ntff_hook_package():
    import os
    site = "/root/.axon_site"
    try:
        pkg = os.path.join(site, "antenv")
        os.makedirs(pkg, exist_ok=True)
        init = os.path.join(pkg, "__init__.py")
        if not os.path.exists(init):
            with open(init, "w") as f:
                f.write("import pkgutil\n__path__ = pkgutil.extend_path(__path__, __name__)\n")
        hooks = os.path.join(pkg, "axon_hooks.py")
        if not os.path.exists(hooks):
            with open(hooks, "w") as f:
                f.write(
                    "_H = None\n"
                    "def set_axon_ntff_profile_hook(h):\n"
                    "    global _H\n"
                    "    _H = h\n"
                    "def get_axon_ntff_profile_hook():\n"
                    "    return _H\n")
    except Exception:
        pass


_ensure_ntff_hook_package()

_CACHE = {}
LAST_EXEC_NS = None


def _run(inputs, trace=True):
    x = np.asarray(inputs["x"], np.float32)
    edge_index = np.asarray(inputs["edge_index"])
    cfg = Cfg(x.shape[0])
    weights = {k: v for k, v in inputs.items() if k not in ("x", "edge_index")}
    in_maps, layout, ln_identity = preprocess(cfg, x, edge_index, weights)

    key = (x.shape, edge_index.shape, layout["NT"], layout["COLS"], ln_identity,
           SPLIT_AG, WT, GCAP, OHW, DMA_SCRATCH)
    if key in _CACHE:
        nc = _CACHE[key]
    else:
        nc = build_program(cfg, layout, ln_identity)
        _CACHE[key] = nc

    from concourse.bass_utils import run_bass_kernel_spmd
    import concourse.bass_utils as bu
    bu.upload_artifacts = lambda d: d
    res = run_bass_kernel_spmd(nc, in_maps, core_ids=list(range(NCORES)),
                               trace=trace)
    outs = [res.results[c]["out"] for c in range(NCORES)]
    out = np.concatenate([np.asarray(o)[:cfg.NPC] for o in outs])
    return out.astype(np.float32), res.exec_time_ns


def kernel(**inputs):
    global LAST_EXEC_NS
    try:
        out, ns = _run(inputs, trace=True)
        LAST_EXEC_NS = ns
        return out
    except Exception:
        out, _ = _run(inputs, trace=False)
        LAST_EXEC_NS = None
        return out


if __name__ == "__main__":
    d = np.load('/tmp/ref_data.npz')
    inputs = {k[3:]: d[k] for k in d.files if k.startswith('in_')}
    cfg = Cfg(np.asarray(inputs['x']).shape[0])
    weights = {k: v for k, v in inputs.items() if k not in ('x', 'edge_index')}
    in_maps, layout, lnid = preprocess(
        cfg, np.asarray(inputs['x'], np.float32),
        np.asarray(inputs['edge_index']), weights)
    print("NTq:", layout["NTq"], "NT:", layout["NT"], "SLOTS:", layout["SLOTS"])
    print("COLS:", layout["COLS"], "MAXSPAN:", layout["MAXSPAN"],
          "mean cols_b:", np.mean(layout["cols_b"]))
    print("calls:", len(layout["calls"]))
    E = np.asarray(inputs['edge_index']).shape[1]
    print("slots/core vs E/8:", layout["SLOTS"], E / 8,
          "pad frac:", 1 - E / 8 / layout["SLOTS"])



# revision 32
# speedup vs baseline: 1.3282x; 1.1599x over previous
"""Self-contained Trainium2 Bass kernel for the 4-layer GraphSAGE GNN
(nn_EnhancedClassifier): kernel(**inputs) -> np.ndarray [100000] f32.

Runs SPMD on 8 NeuronCores via run_bass_kernel_spmd.

v2 strategy: dst-partition nodes across 8 cores. Per core, edges are
sorted by (src-quarter stream, dst_block) and packed into 4 per-chunk
tile streams with unit-anchored scheduling: groups of UNIT dst blocks
share a start offset = cumulative max edge count over cores (keeps the
SPMD program uniform with only ~5% pad, vs 25% for per-block max), and
tiles at block boundaries are shared by adjacent blocks (the one-hot
zeroes foreign edges). x is zero-padded to 128 feats bf16 so all 4
layers gather 256B rows. h_full is split into 4 per-quarter Shared DRAM
tensors so each AllGather quarter unblocks that chunk's gathers early.
The own-path h stays resident in SBUF between layers; PSUM->SBUF copies
run on the Scalar engine to keep DVE free for one-hot builds.
"""
import sys
sys.path.insert(0, '/opt/trn_rl_repo')
import numpy as np
import ml_dtypes
from concourse import bass, bacc, mybir, tile

BF16 = mybir.dt.bfloat16
F32 = mybir.dt.float32
FP8 = mybir.dt.float8e4
AF = mybir.ActivationFunctionType
ALU = mybir.AluOpType

NCORES = 8

# --- Patch Tile's DMASW lane assignment to be SWDGE-queue-aware: lane%4 must
# equal the instruction's queue_num or the runtime rejects the sem update.
import concourse.tile_sem_assignment as _tsa
from concourse import bass_isa as _bisa

if not getattr(_tsa, "_gnn_queue_patch", False):
    _orig_assign_tick = _tsa.TileClockTick._assign_tick

    def _assign_tick_qaware(self, inst):
        if isinstance(inst, mybir.InstDMAGatherAnt):
            q = inst.queue_num
            rot = self.__dict__.setdefault("_gnn_qrot", {})
            k = rot.get(q, 0)
            rot[q] = k ^ 1
            self.next_sw_dma_idx = q + 4 * k
        elif (isinstance(inst, _tsa.DMAInst)
              and inst.engine == mybir.EngineType.Pool
              and not isinstance(inst, _bisa.UserSyncedRemoteDMADescs)):
            rot = self.__dict__.setdefault("_gnn_qrot", {})
            k = rot.get(0, 0)
            rot[0] = k ^ 1
            self.next_sw_dma_idx = 4 * k
        return _orig_assign_tick(self, inst)

    _tsa.TileClockTick._assign_tick = _assign_tick_qaware
    _tsa._gnn_queue_patch = True

IN_F = 64
HID = 128
PAD_DSTLOC = 1000.0
NQ = 4              # chunk streams / AllGather quarters
import os as _os
WT = int(_os.environ.get("GNN_WT", "8"))   # tiles per dma_gather call
SPLIT_AG = int(_os.environ.get("GNN_SPLIT_AG", "1"))   # per-quarter AllGathers
# one-hot micro-block width: edges are dst-sorted into W-wide micro blocks;
# each one-hot matmul is [128 slots x W dsts], so one-hot DMA bytes scale
# with W (W=64 halves them vs 128). 128/W micro blocks share one PSUM tile.
OHW = int(_os.environ.get("GNN_OHW", "64"))
UNIT = 128 // OHW   # micro blocks per anchored unit (= per macro block)
# one-hot DMA batching: micro blocks are grouped until their span-columns
# reach GCAP (bytes/partition = GCAP*OHW), fetched with ONE dma_start.
GCAP = int(_os.environ.get("GNN_GCAP", str(12288 // OHW)))
# SWDGE descriptor-ring carveout (bytes/partition of SBUF). The default 16KB
# gives each (queue, DMA-engine) ring 64 descriptors -- LESS than one WT=8
# gather call (66), so calls serialize on ring space. 48KB lets ~3 calls per
# queue be in flight.
DMA_SCRATCH = int(_os.environ.get("GNN_DMA_SCRATCH", "49152"))


class Cfg:
    def __init__(self, n_nodes):
        self.N = n_nodes
        self.NPC = n_nodes // NCORES
        assert self.NPC * NCORES == self.N
        self.B = (self.NPC + 127) // 128             # blocks per core
        self.ROWS = self.B * 128                     # padded rows per core
        self.GROWS = self.ROWS * NCORES
        # block-aligned quarters of each core's rows
        bq = self.B // NQ
        extra = self.B - bq * NQ
        self.qblocks = [bq + (1 if k < extra else 0) for k in range(NQ)]
        self.qrows = [q * 128 for q in self.qblocks]
        self.qstart = np.concatenate([[0], np.cumsum(self.qrows)]).astype(np.int64)
        self.chunk_rows = [NCORES * r for r in self.qrows]
        assert max(self.chunk_rows) <= 32768, "idx must fit int16"


def preprocess(cfg, x, edge_index, weights):
    src = edge_index[0].astype(np.int64)
    dst = edge_index[1].astype(np.int64)
    N, B = cfg.N, cfg.B

    deg = np.bincount(dst, minlength=N).astype(np.float32)
    deginv = 1.0 / np.maximum(deg, 1.0)

    s_core = src // cfg.NPC
    s_loc = src % cfg.NPC
    s_q = np.searchsorted(cfg.qstart[1:], s_loc, side='right')
    qrows_a = np.asarray(cfg.qrows, np.int64)
    s_row = s_core * qrows_a[s_q] + (s_loc - cfg.qstart[s_q])   # chunk-relative

    d_core = dst // cfg.NPC
    d_loc = dst % cfg.NPC
    BM = B * UNIT                 # micro blocks (OHW dsts each) per core
    d_block = d_loc // OHW        # micro block index

    # within each (core, stream, micro-block) run, order slots by ascending
    # src row: the one-hot encodes slot->dst anyway, and ascending gather
    # addresses give the HBM better locality than dst-sorted (random) reads.
    order = np.lexsort((s_row, d_block, s_q, d_core))
    s_row_s = s_row[order]
    d_loc_s = d_loc[order]
    key = (d_core[order] * NQ + s_q[order]) * BM + d_block[order]
    gstart = np.searchsorted(key, np.arange(NCORES * NQ * BM + 1))

    # per (core, q, micro) counts
    cnt = (gstart[1:] - gstart[:-1]).reshape(NCORES, NQ, BM)

    # ---- unit-anchored stream scheduling (uniform across cores); one unit
    # = one macro block (128 dsts) = UNIT micro blocks sharing slot slack
    NU = B
    S = np.zeros((NQ, NU + 1), np.int64)            # unit start slots per stream
    ucnt_max = np.zeros((NQ, NU), np.int64)
    for q in range(NQ):
        for u in range(NU):
            b0, b1 = u * UNIT, min(BM, (u + 1) * UNIT)
            ucnt_max[q, u] = cnt[:, q, b0:b1].sum(axis=1).max()
        S[q, 1:] = np.cumsum(ucnt_max[q])
    NTq = [int((S[q, NU] + 127) // 128) for q in range(NQ)]
    qtile0 = np.concatenate([[0], np.cumsum(NTq)]).astype(np.int64)
    NT = int(qtile0[-1])
    SLOTS = NT * 128

    # per-core micro-block positions within streams
    pos0 = np.zeros((NCORES, NQ, BM), np.int64)
    pos1 = np.zeros((NCORES, NQ, BM), np.int64)
    for q in range(NQ):
        for u in range(NU):
            b0, b1 = u * UNIT, min(BM, (u + 1) * UNIT)
            run = np.cumsum(
                np.concatenate([np.zeros((NCORES, 1), np.int64),
                                cnt[:, q, b0:b1]], axis=1), axis=1)
            pos0[:, q, b0:b1] = S[q, u] + run[:, :-1]
            pos1[:, q, b0:b1] = S[q, u] + run[:, 1:]

    # union spans per (micro, q) across cores
    uspans = []
    for m in range(BM):
        sp = []
        for q in range(NQ):
            mask = pos1[:, q, m] > pos0[:, q, m]
            if mask.any():
                t0 = int(pos0[mask, q, m].min() // 128)
                t1 = int(-(-pos1[mask, q, m].max() // 128))
                sp.append((q, t0, t1))
        uspans.append(sp)
    cols_b = [sum(t1 - t0 for (_, t0, t1) in uspans[m]) for m in range(BM)]
    COLS = int(sum(cols_b))
    col0_b = np.concatenate([[0], np.cumsum(cols_b)]).astype(np.int64)
    MAXSPAN = max(cols_b)

    # gather calls: interleave streams window-major
    calls = []
    for lo in range(0, max(NTq), WT):
        for q in range(NQ):
            if lo < NTq[q]:
                calls.append((q, lo, min(WT, NTq[q] - lo)))

    # one-hot load groups: consecutive micros packed until GCAP span-columns
    groups = []           # (m0, nmicros, col0, ncols)
    blk2grp = np.zeros(BM, np.int64)
    m = 0
    while m < BM:
        m0, ctot = m, 0
        while m < BM and (m == m0 or ctot + cols_b[m] <= GCAP):
            ctot += cols_b[m]
            blk2grp[m] = len(groups)
            m += 1
        groups.append((m0, m - m0, int(col0_b[m0]), ctot))
    assert max(g[3] for g in groups) <= max(GCAP, max(cols_b))

    layout = dict(NTq=NTq, qtile0=qtile0, NT=NT, SLOTS=SLOTS, uspans=uspans,
                  cols_b=cols_b, col0_b=col0_b, COLS=COLS, MAXSPAN=MAXSPAN,
                  calls=calls, groups=groups, blk2grp=blk2grp)

    # ---- weights / constants
    W = {k: np.asarray(v) for k, v in weights.items()}

    def padw(w):
        out = np.zeros((HID, HID), np.float32)
        out[:w.shape[0], :w.shape[1]] = w
        return out.astype(ml_dtypes.bfloat16)

    wcast = {
        "Wl1": padw(W["Wl1"]), "Wr1": padw(W["Wr1"]), "Wres": padw(W["Wres"]),
        "Wl2": W["Wl2"].astype(ml_dtypes.bfloat16),
        "Wr2": W["Wr2"].astype(ml_dtypes.bfloat16),
        "Wl3": W["Wl3"].astype(ml_dtypes.bfloat16),
        "Wr3": W["Wr3"].astype(ml_dtypes.bfloat16),
        "Wl4": W["Wl4"].astype(ml_dtypes.bfloat16),
        "Wr4": W["Wr4"].astype(ml_dtypes.bfloat16),
    }
    brow = {k: W[k].reshape(1, -1).astype(ml_dtypes.bfloat16)
            for k in ["b1", "bres", "b2", "b3", "b4"]}
    ln_g = W["ln_g"].astype(np.float32)
    ln_b = W["ln_b"].astype(np.float32)
    ln_identity = bool(np.all(ln_g == 1.0) and np.all(ln_b == 0.0))
    ln_g_rep = np.broadcast_to(ln_g, (128, HID)).copy()
    ln_b_rep = np.broadcast_to(ln_b, (128, HID)).copy()

    iota_bf = np.broadcast_to(np.arange(128, dtype=np.float32), (128, 128)).astype(
        ml_dtypes.bfloat16).copy()
    ones_row = np.ones((1, 128), ml_dtypes.bfloat16)
    ident_bf = np.eye(128, dtype=ml_dtypes.bfloat16)

    # x zero-padded to 128 feats bf16, laid out [chunk q][core][qrows]
    x_cast = np.asarray(x, np.float32).astype(ml_dtypes.bfloat16)
    xq = []
    for q in range(NQ):
        xb = np.zeros((cfg.chunk_rows[q], HID), ml_dtypes.bfloat16)
        for c in range(NCORES):
            lo = c * cfg.NPC + cfg.qstart[q]
            n = int(min(cfg.qrows[q], max(0, cfg.NPC - cfg.qstart[q])))
            if n > 0:
                xb[c * cfg.qrows[q]: c * cfg.qrows[q] + n, :IN_F] = x_cast[lo:lo + n]
        xq.append(xb)

    # ---- per-core inputs
    in_maps = []
    for c in range(NCORES):
        idx_lin = np.zeros(SLOTS, np.int16)
        for q in range(NQ):
            g0 = (c * NQ + q) * BM
            for mb in range(BM):
                lo_e, hi_e = gstart[g0 + mb], gstart[g0 + mb + 1]
                n = int(hi_e - lo_e)
                if n == 0:
                    continue
                s0 = int(pos0[c, q, mb])
                gslot = qtile0[q] * 128 + s0
                idx_lin[gslot:gslot + n] = s_row_s[lo_e:hi_e].astype(np.int16)
        # host-built one-hot, fp8: oh_all[p, col*OHW + j] = 1 iff the edge at
        # slot (tile(col), p) belongs to micro(col) and has dst j (local)
        oh_all = np.zeros((128, COLS * OHW), ml_dtypes.float8_e4m3fn)
        col = 0
        for mb in range(BM):
            for (q, t0, t1) in uspans[mb]:
                g0 = (c * NQ + q) * BM
                lo_e, hi_e = gstart[g0 + mb], gstart[g0 + mb + 1]
                n = int(hi_e - lo_e)
                if n:
                    s0 = int(pos0[c, q, mb])
                    dl = d_loc_s[lo_e:hi_e] - mb * OHW
                    sl = np.arange(s0, s0 + n)
                    tt = sl // 128 - t0
                    pp = sl % 128
                    oh_all[pp, (col + tt) * OHW + dl] = 1.0
                col += (t1 - t0)
        assert col == COLS

        idx_pk = idx_lin.reshape(SLOTS // 16, 16).T
        idx_pk = np.tile(idx_pk, (8, 1))

        dinv_col = np.ones((128, B), np.float32)
        basec = c * cfg.NPC
        for b in range(B):
            n_real = min(128, cfg.NPC - b * 128)
            dinv_col[:n_real, b] = deginv[basec + b * 128: basec + b * 128 + n_real]

        x_own = np.zeros((cfg.ROWS, HID), ml_dtypes.bfloat16)
        x_own[:cfg.NPC, :IN_F] = x_cast[c * cfg.NPC:(c + 1) * cfg.NPC]

        m = {
            "idx16": np.ascontiguousarray(idx_pk),
            "oh_all": oh_all,
            "deginv": dinv_col,
            "x_own": x_own,
            "ones_row": ones_row,
            "ident": ident_bf,
            "ln_g_rep": ln_g_rep,
            "ln_b_rep": ln_b_rep,
        }
        for q in range(NQ):
            m[f"x_q{q}"] = xq[q]
        m.update(wcast)
        m.update(brow)
        in_maps.append(m)

    return in_maps, layout, ln_identity


def build_program(cfg, layout, ln_identity):
    B, ROWS = cfg.B, cfg.ROWS
    COLS, SLOTS = layout["COLS"], layout["SLOTS"]
    uspans, col0_b = layout["uspans"], layout["col0_b"]
    qtile0, calls = layout["qtile0"], layout["calls"]
    MAXSPAN = layout["MAXSPAN"]
    nc = bacc.Bacc("TRN2", target_bir_lowering=False, debug=False,
                   num_devices=NCORES, num_swdge_queues=4,
                   dynamic_dma_scratch_size=DMA_SCRATCH)

    x_q = [nc.dram_tensor(f"x_q{q}", [cfg.chunk_rows[q], HID], BF16,
                          kind="ExternalInput") for q in range(NQ)]
    x_own = nc.dram_tensor("x_own", [ROWS, HID], BF16, kind="ExternalInput")
    idx_d = nc.dram_tensor("idx16", [128, SLOTS // 16], mybir.dt.int16,
                           kind="ExternalInput")
    oh_d = nc.dram_tensor("oh_all", [128, COLS * OHW], FP8, kind="ExternalInput")
    deginv_d = nc.dram_tensor("deginv", [128, B], F32, kind="ExternalInput")
    ones_d = nc.dram_tensor("ones_row", [1, 128], BF16, kind="ExternalInput")
    ident_d = nc.dram_tensor("ident", [128, 128], BF16, kind="ExternalInput")
    lng_d = nc.dram_tensor("ln_g_rep", [128, HID], F32, kind="ExternalInput")
    lnb_d = nc.dram_tensor("ln_b_rep", [128, HID], F32, kind="ExternalInput")
    wd = {k: nc.dram_tensor(k, [HID, HID], BF16, kind="ExternalInput")
          for k in ["Wl1", "Wr1", "Wres", "Wl2", "Wr2", "Wl3", "Wr3"]}
    wd["Wl4"] = nc.dram_tensor("Wl4", [HID, 1], BF16, kind="ExternalInput")
    wd["Wr4"] = nc.dram_tensor("Wr4", [HID, 1], BF16, kind="ExternalInput")
    bd = {k: nc.dram_tensor(k, [1, HID], BF16, kind="ExternalInput")
          for k in ["b1", "bres", "b2", "b3"]}
    bd["b4"] = nc.dram_tensor("b4", [1, 1], BF16, kind="ExternalInput")

    out_d = nc.dram_tensor("out", [ROWS], F32, kind="ExternalOutput")
    rg = [list(range(NCORES))]

    with tile.TileContext(nc) as tc:
        with (
            tc.tile_pool(name="dramp", bufs=1, space="DRAM") as dramp,
            tc.tile_pool(name="const", bufs=1) as constp,
            tc.tile_pool(name="meta", bufs=1) as metap,
            tc.tile_pool(name="gpool", bufs=max(10, 80 // WT)) as gpool,
            tc.tile_pool(name="ohpool", bufs=3) as ohpool,
            tc.tile_pool(name="spool", bufs=4) as spool,
            tc.tile_pool(name="hres", bufs=1) as hresp,
            tc.tile_pool(name="outp", bufs=1) as outp,
            tc.tile_pool(name="ps", bufs=2, space="PSUM") as ps,
        ):
            if SPLIT_AG:
                # per-quarter local h and Shared gathered h (3 layers x NQ)
                h_own = [[dramp.tile([cfg.qrows[k], HID], BF16,
                                     tag=f"h_own{l}_{k}", name=f"h_own{l}_{k}")
                          for k in range(NQ)] for l in range(3)]
                h_full = [[dramp.tile([cfg.chunk_rows[k], HID], BF16,
                                      tag=f"h_full{l}_{k}", name=f"h_full{l}_{k}",
                                      addr_space="Shared") for k in range(NQ)]
                          for l in range(3)]
            else:
                h_own1 = [dramp.tile([ROWS, HID], BF16, tag=f"h_own{l}",
                                     name=f"h_own{l}") for l in range(3)]
                h_full1 = [dramp.tile([cfg.GROWS, HID], BF16, tag=f"h_full{l}",
                                      name=f"h_full{l}", addr_space="Shared")
                           for l in range(3)]
            coff = np.concatenate(
                [[0], np.cumsum(cfg.chunk_rows)]).astype(np.int64)

            def h_src_aps(l):
                if SPLIT_AG:
                    return [h_full[l][k][:] for k in range(NQ)]
                return [h_full1[l][int(coff[k]):int(coff[k + 1]), :]
                        for k in range(NQ)]

            idx_t = metap.tile([128, SLOTS // 16], mybir.dt.int16)
            nc.sync.dma_start(out=idx_t[:], in_=idx_d[:])
            deginv_t = metap.tile([128, B], F32)
            nc.sync.dma_start(out=deginv_t[:], in_=deginv_d[:])
            ones_t = constp.tile([1, 128], BF16)
            nc.sync.dma_start(out=ones_t[:], in_=ones_d[:])
            ident_t = constp.tile([128, 128], BF16)
            nc.sync.dma_start(out=ident_t[:], in_=ident_d[:])
            eps_t = constp.tile([128, 1], F32)
            nc.vector.memset(eps_t[:], 1e-5)
            lng_t = constp.tile([128, HID], F32)
            nc.sync.dma_start(out=lng_t[:], in_=lng_d[:])
            lnb_t = constp.tile([128, HID], F32)
            nc.sync.dma_start(out=lnb_t[:], in_=lnb_d[:])
            w_t = {}
            for k, h in wd.items():
                w_t[k] = constp.tile(list(h.shape), BF16, tag=f"w_{k}", name=f"w_{k}")
                nc.sync.dma_start(out=w_t[k][:], in_=h[:])
            b_t = {}
            for k, h in bd.items():
                b_t[k] = constp.tile(list(h.shape), BF16, tag=f"b_{k}", name=f"b_{k}")
                nc.sync.dma_start(out=b_t[k][:], in_=h[:])

            out_sb = outp.tile([128, B], F32)
            hsb = [hresp.tile([128, B * HID], BF16, tag=f"hsb{i}",
                              name=f"hsb{i}") for i in range(2)]

            state = {"layer": 0}

            def new_layer(src_list):
                state["src"] = src_list
                state["G"] = {}
                state["nxt"] = 0
                state["cov"] = [0] * NQ
                state["layer"] += 1

            def issue_calls_until(need_q, need_t1):
                while state["cov"][need_q] < need_t1:
                    k = state["nxt"]
                    assert k < len(calls), (need_q, need_t1, state["cov"])
                    q, lo, nt = calls[k]
                    G = gpool.tile([128, WT * HID], BF16, tag="G",
                                   name=f"G_{state['layer']}_{k}")
                    base16 = (int(qtile0[q]) + lo) * 8
                    rows = nt * 128
                    nc.gpsimd.dma_gather(
                        out_ap=G[:, :nt * HID].rearrange(
                            "p (t e) -> p t e", e=HID),
                        in_ap=state["src"][q],
                        idxs_ap=idx_t[:, base16:base16 + rows // 16],
                        num_idxs=rows,
                        num_idxs_reg=rows,
                        elem_size=HID,
                        queue_num=k % 4,
                    )
                    state["G"][(q, lo // WT)] = G
                    state["cov"][q] = lo + nt
                    state["nxt"] = k + 1

            groups, blk2grp = layout["groups"], layout["blk2grp"]
            GMAX = max(g[3] for g in groups)

            def build_onehot(mb):
                # batched one-hot fetch: one dma_start per micro-block group
                nb = layout["cols_b"][mb]
                g = int(blk2grp[mb])
                m0, nbk, c0, ncols = groups[g]
                if mb == m0 and ncols > 0:
                    oh = ohpool.tile([128, GMAX * OHW], FP8, tag="oh")
                    nc.sync.dma_start(out=oh[:, :ncols * OHW],
                                      in_=oh_d[:, c0 * OHW:(c0 + ncols) * OHW])
                    state["ohg"] = oh
                if nb == 0:
                    return None, 0
                return state["ohg"], int(col0_b[mb]) - c0

            def scatter(b, agg_psum):
                # aggregate macro block b = UNIT micro blocks of OHW dsts,
                # each accumulating into its own PSUM column slice
                any_mm = False
                for j in range(UNIT):
                    mb = b * UNIT + j
                    for (q, t0, t1) in uspans[mb]:
                        issue_calls_until(q, t1)
                    oh, coff = build_onehot(mb)
                    sub = agg_psum[:, j * OHW:(j + 1) * OHW]
                    if oh is None:
                        nc.vector.memset(sub, 0.0)
                        continue
                    any_mm = True
                    n_mm = sum(t1 - t0 for (_, t0, t1) in uspans[mb])
                    col = coff
                    for (q, t0, t1) in uspans[mb]:
                        for t in range(t0, t1):
                            G = state["G"][(q, t // WT)]
                            off = (t % WT) * HID
                            nc.tensor.matmul(
                                sub, lhsT=G[:, off:off + HID],
                                rhs=oh[:, col * OHW:(col + 1) * OHW],
                                start=(col == coff),
                                stop=(col == coff + n_mm - 1))
                            col += 1
                return any_mm

            qlast = np.cumsum(cfg.qblocks) - 1      # last block of each quarter

            def maybe_allgather(l, b):
                if SPLIT_AG:
                    for k in range(NQ):
                        if b == qlast[k]:
                            nc.gpsimd.collective_compute(
                                "AllGather", ALU.bypass, replica_groups=rg,
                                ins=[h_own[l][k][:]], outs=[h_full[l][k][:]])
                elif b == B - 1:
                    nc.gpsimd.collective_compute(
                        "AllGather", ALU.bypass, replica_groups=rg,
                        ins=[h_own1[l][:]], outs=[h_full1[l][:]])

            def write_h(l, b, src_ap):
                if SPLIT_AG:
                    k = int(np.searchsorted(qlast, b))
                    b0 = int(qlast[k]) - cfg.qblocks[k] + 1
                    ro = (b - b0) * 128
                    nc.sync.dma_start(out=h_own[l][k][ro:ro + 128, :], in_=src_ap)
                else:
                    nc.sync.dma_start(
                        out=h_own1[l][b * 128:(b + 1) * 128, :], in_=src_ap)

            # =================== Layer 1 ===================
            new_layer([x_q[q][:] for q in range(NQ)])
            for b in range(B):
                xblk = spool.tile([128, HID], BF16, tag="hblk")
                nc.sync.dma_start(out=xblk[:], in_=x_own[b * 128:(b + 1) * 128, :])
                xT_ps = ps.tile([HID, 128], BF16, tag="xT_ps", bufs=1)
                nc.tensor.transpose(xT_ps[:], xblk[:], ident_t[:])
                xT = spool.tile([HID, 128], BF16, tag="hT")
                nc.scalar.activation(xT[:], xT_ps[:], AF.Copy)

                agg_ps = ps.tile([HID, 128], F32, tag="agg", bufs=2)
                has_agg = scatter(b, agg_ps)
                aggT = spool.tile([HID, 128], BF16, tag="aggT")
                if has_agg:
                    nc.scalar.activation(aggT[:], agg_ps[:], AF.Copy)
                else:
                    nc.vector.memset(aggT[:], 0.0)

                zA = ps.tile([128, HID], F32, tag="zA", bufs=2)
                nc.tensor.matmul(zA[:], lhsT=aggT[:], rhs=w_t["Wl1"][:],
                                 start=True, stop=True)
                zB = ps.tile([128, HID], F32, tag="zB", bufs=2)
                nc.tensor.matmul(zB[:], lhsT=xT[:], rhs=w_t["Wr1"][:],
                                 start=True, stop=False)
                nc.tensor.matmul(zB[:], lhsT=ones_t[:], rhs=b_t["b1"][:],
                                 start=False, stop=True)
                res = ps.tile([128, HID], F32, tag="res", bufs=1)
                nc.tensor.matmul(res[:], lhsT=xT[:], rhs=w_t["Wres"][:],
                                 start=True, stop=False)
                nc.tensor.matmul(res[:], lhsT=ones_t[:], rhs=b_t["bres"][:],
                                 start=False, stop=True)

                sA = spool.tile([128, HID], F32, tag="sA")
                nc.vector.tensor_scalar(
                    out=sA[:], in0=zA[:], scalar1=deginv_t[:, b:b + 1],
                    scalar2=None, op0=ALU.mult)
                z = spool.tile([128, HID], F32, tag="z")
                nc.vector.tensor_tensor(out=z[:], in0=sA[:], in1=zB[:], op=ALU.add)

                # LayerNorm via fused bn_stats: one DVE pass for mean+var,
                # then one Scalar pass Relu(z*rstd - mu*rstd)
                st6 = spool.tile([128, 6], F32, tag="st6")
                nc.vector.bn_stats(st6[:], z[:])
                agr = spool.tile([128, 2], F32, tag="agr")
                nc.vector.bn_aggr(agr[:], st6[:])
                std = spool.tile([128, 1], F32, tag="std")
                nc.scalar.activation(std[:], agr[:, 1:2], AF.Sqrt, bias=eps_t[:])
                rstd = spool.tile([128, 1], F32, tag="rstd")
                nc.vector.reciprocal(rstd[:], std[:])
                nmu = spool.tile([128, 1], F32, tag="nmu")
                nc.vector.tensor_scalar(out=nmu[:], in0=agr[:, 0:1],
                                        scalar1=rstd[:], scalar2=-1.0,
                                        op0=ALU.mult, op1=ALU.mult)

                if ln_identity:
                    zr = spool.tile([128, HID], F32, tag="zr")
                    nc.scalar.activation(zr[:], z[:], AF.Relu, bias=nmu[:],
                                         scale=rstd[:])
                else:
                    zn = spool.tile([128, HID], F32, tag="zn")
                    nc.scalar.activation(zn[:], z[:], AF.Identity, bias=nmu[:],
                                         scale=rstd[:])
                    nc.vector.tensor_tensor(out=zn[:], in0=zn[:], in1=lng_t[:],
                                            op=ALU.mult)
                    nc.vector.tensor_tensor(out=zn[:], in0=zn[:], in1=lnb_t[:],
                                            op=ALU.add)
                    zr = spool.tile([128, HID], F32, tag="zr")
                    nc.vector.tensor_scalar(out=zr[:], in0=zn[:], scalar1=0.0,
                                            scalar2=None, op0=ALU.max)

                nc.vector.tensor_tensor(out=hsb[0][:, b * HID:(b + 1) * HID],
                                        in0=zr[:], in1=res[:], op=ALU.add)
                write_h(0, b, hsb[0][:, b * HID:(b + 1) * HID])
                maybe_allgather(0, b)

            # =================== Layers 2,3 ===================
            for li, (wl, wr, bb) in enumerate(
                    [("Wl2", "Wr2", "b2"), ("Wl3", "Wr3", "b3")]):
                new_layer(h_src_aps(li))
                hprev = hsb[li % 2]
                hcur = hsb[(li + 1) % 2]
                for b in range(B):
                    hT_ps = ps.tile([HID, 128], BF16, tag="xT_ps", bufs=1)
                    nc.tensor.transpose(hT_ps[:], hprev[:, b * HID:(b + 1) * HID],
                                        ident_t[:])
                    hT = spool.tile([HID, 128], BF16, tag="hT")
                    nc.scalar.activation(hT[:], hT_ps[:], AF.Copy)
                    agg_ps = ps.tile([HID, 128], F32, tag="agg", bufs=2)
                    has_agg = scatter(b, agg_ps)
                    aggT = spool.tile([HID, 128], BF16, tag="aggT")
                    if has_agg:
                        nc.scalar.activation(aggT[:], agg_ps[:], AF.Copy)
                    else:
                        nc.vector.memset(aggT[:], 0.0)

                    zA = ps.tile([128, HID], F32, tag="zA", bufs=2)
                    nc.tensor.matmul(zA[:], lhsT=aggT[:], rhs=w_t[wl][:],
                                     start=True, stop=True)
                    zB = ps.tile([128, HID], F32, tag="zB", bufs=2)
                    nc.tensor.matmul(zB[:], lhsT=hT[:], rhs=w_t[wr][:],
                                     start=True, stop=False)
                    nc.tensor.matmul(zB[:], lhsT=ones_t[:], rhs=b_t[bb][:],
                                     start=False, stop=True)

                    sA = spool.tile([128, HID], F32, tag="sA")
                    nc.vector.tensor_scalar(
                        out=sA[:], in0=zA[:], scalar1=deginv_t[:, b:b + 1],
                        scalar2=None, op0=ALU.mult)
                    z = spool.tile([128, HID], F32, tag="z")
                    nc.vector.tensor_tensor(out=z[:], in0=sA[:], in1=zB[:],
                                            op=ALU.add)
                    nc.scalar.activation(hcur[:, b * HID:(b + 1) * HID],
                                         z[:], AF.Relu)
                    write_h(li + 1, b, hcur[:, b * HID:(b + 1) * HID])
                    maybe_allgather(li + 1, b)

            # =================== Layer 4 ===================
            new_layer(h_src_aps(2))
            hprev = hsb[0]
            for b in range(B):
                hT_ps = ps.tile([HID, 128], BF16, tag="xT_ps", bufs=1)
                nc.tensor.transpose(hT_ps[:], hprev[:, b * HID:(b + 1) * HID],
                                    ident_t[:])
                hT = spool.tile([HID, 128], BF16, tag="hT")
                nc.scalar.activation(hT[:], hT_ps[:], AF.Copy)
                agg_ps = ps.tile([HID, 128], F32, tag="agg", bufs=2)
                has_agg = scatter(b, agg_ps)
                aggT = spool.tile([HID, 128], BF16, tag="aggT")
                if has_agg:
                    nc.scalar.activation(aggT[:], agg_ps[:], AF.Copy)
                else:
                    nc.vector.memset(aggT[:], 0.0)

                oA = ps.tile([128, 1], F32, tag="zA", bufs=2)
                nc.tensor.matmul(oA[:], lhsT=aggT[:], rhs=w_t["Wl4"][:],
                                 start=True, stop=True)
                oB = ps.tile([128, 1], F32, tag="zB", bufs=2)
                nc.tensor.matmul(oB[:], lhsT=hT[:], rhs=w_t["Wr4"][:],
                                 start=True, stop=False)
                nc.tensor.matmul(oB[:], lhsT=ones_t[:], rhs=b_t["b4"][:],
                                 start=False, stop=True)
                t4 = spool.tile([128, 1], F32, tag="t4")
                nc.vector.tensor_scalar(
                    out=t4[:], in0=oA[:], scalar1=deginv_t[:, b:b + 1],
                    scalar2=None, op0=ALU.mult)
                nc.vector.tensor_tensor(out=out_sb[:, b:b + 1], in0=t4[:],
                                        in1=oB[:], op=ALU.add)

            nc.sync.dma_start(
                out=out_d[:].rearrange("(b p) -> p b", p=128), in_=out_sb[:])

    nc.compile()
    return nc


# ---------------------------------------------------------------------------
# Self-contained entry point


def _ensure_ntff_hook_package():
    import os
    site = "/root/.axon_site"
    try:
        pkg = os.path.join(site, "antenv")
        os.makedirs(pkg, exist_ok=True)
        init = os.path.join(pkg, "__init__.py")
        if not os.path.exists(init):
            with open(init, "w") as f:
                f.write("import pkgutil\n__path__ = pkgutil.extend_path(__path__, __name__)\n")
        hooks = os.path.join(pkg, "axon_hooks.py")
        if not os.path.exists(hooks):
            with open(hooks, "w") as f:
                f.write(
                    "_H = None\n"
                    "def set_axon_ntff_profile_hook(h):\n"
                    "    global _H\n"
                    "    _H = h\n"
                    "def get_axon_ntff_profile_hook():\n"
                    "    return _H\n")
    except Exception:
        pass


_ensure_ntff_hook_package()

_CACHE = {}
LAST_EXEC_NS = None


def _run(inputs, trace=True):
    x = np.asarray(inputs["x"], np.float32)
    edge_index = np.asarray(inputs["edge_index"])
    cfg = Cfg(x.shape[0])
    weights = {k: v for k, v in inputs.items() if k not in ("x", "edge_index")}
    in_maps, layout, ln_identity = preprocess(cfg, x, edge_index, weights)

    key = (x.shape, edge_index.shape, layout["NT"], layout["COLS"], ln_identity,
           SPLIT_AG, WT, GCAP, OHW, DMA_SCRATCH)
    if key in _CACHE:
        nc = _CACHE[key]
    else:
        nc = build_program(cfg, layout, ln_identity)
        _CACHE[key] = nc

    from concourse.bass_utils import run_bass_kernel_spmd
    import concourse.bass_utils as bu
    bu.upload_artifacts = lambda d: d
    res = run_bass_kernel_spmd(nc, in_maps, core_ids=list(range(NCORES)),
                               trace=trace)
    outs = [res.results[c]["out"] for c in range(NCORES)]
    out = np.concatenate([np.asarray(o)[:cfg.NPC] for o in outs])
    return out.astype(np.float32), res.exec_time_ns


def kernel(**inputs):
    global LAST_EXEC_NS
    try:
        out, ns = _run(inputs, trace=True)
        LAST_EXEC_NS = ns
        return out
    except Exception:
        out, _ = _run(inputs, trace=False)
        LAST_EXEC_NS = None
        return out


if __name__ == "__main__":
    d = np.load('/tmp/ref_data.npz')
    inputs = {k[3:]: d[k] for k in d.files if k.startswith('in_')}
    cfg = Cfg(np.asarray(inputs['x']).shape[0])
    weights = {k: v for k, v in inputs.items() if k not in ('x', 'edge_index')}
    in_maps, layout, lnid = preprocess(
        cfg, np.asarray(inputs['x'], np.float32),
        np.asarray(inputs['edge_index']), weights)
    print("NTq:", layout["NTq"], "NT:", layout["NT"], "SLOTS:", layout["SLOTS"])
    print("COLS:", layout["COLS"], "MAXSPAN:", layout["MAXSPAN"],
          "mean cols_b:", np.mean(layout["cols_b"]))
    print("calls:", len(layout["calls"]))
    E = np.asarray(inputs['edge_index']).shape[1]
    print("slots/core vs E/8:", layout["SLOTS"], E / 8,
          "pad frac:", 1 - E / 8 / layout["SLOTS"])

